# revision 20
# baseline (speedup 1.0000x reference)
"""Trainium2 Bass kernel for nn_DirectionalMultiHeadClassifier.

Data-parallel over 8 NeuronCores: each core handles 2 of the 16 samples.

Math per sample (mirrors the reference):
  - 4 masked means over S of hidden [S,H]: full attention_mask, and three
    position-range masks derived from L = mask.sum() (first/second/ending).
    Computed on-device as one PSUM-accumulated matmul:
        pooled4[8, H] += W_chunk[128, 8].T @ hidden_chunk[128, H]
    where W is a host-built 0/1 mask matrix (4 mask types x 2 samples) and
    the 1/count normalization is applied afterwards.
  - LayerNorm on the full-mask pooled vector.
  - 4 small MLP heads (H->128 -> exact GELU -> 128->1). The scalar head
    outputs only feed the final classifier's last 4 input features, so the
    128->1 layer is folded into the classifier on the host:
        fc1 += gelu_h @ (0.5 * w2_h outer fc_w1[1024+h, :])
        fc_b1_eff = fc_b1 + sum_h b2_h * fc_w1[1024+h, :]
  - Final classifier (1028->256 -> exact GELU -> 256->5).
  Exact GELU is computed as 0.5*z*(1+erf(z/sqrt(2))) with the 0.5 folded
  into the following layer's weights.
"""

import numpy as np

import concourse.bass as bass
import concourse.tile as tile
from bass_rust import add_dep_helper
from concourse import bacc, mybir
from concourse.bass_utils import run_bass_kernel_spmd

B, S, H = 16, 2048, 1024
NCORES = 8
BPC = B // NCORES          # samples per core
NK = BPC * (S // 128)      # 128-row contraction chunks per core
TS = 1024                  # S rows per hidden DMA tile (4 MiB)
NT = S // TS               # DMA tiles per sample
RS2 = 0.7071067811865476   # 1/sqrt(2)
LN_EPS = 1e-5
EPS = 1e-9
F32 = mybir.dt.float32
HEADS = ["esc", "res", "end", "thr"]

_NC_CACHE = {}


def _build_nc():
    """Build the per-core Bass program (identical on all 8 cores)."""
    from contextlib import ExitStack

    nc = bacc.Bacc(
        "TRN2", target_bir_lowering=False, debug=False, num_devices=NCORES
    )
    dp = nc.declare_dram_parameter
    hid_d = dp("hid", [BPC, S, H], F32, isOutput=False)
    wm_d = dp("wm", [128, NK * 8], F32, isOutput=False)
    invc_d = dp("invc", [8, 1], F32, isOutput=False)
    lng_d = dp("lng", [2, H], F32, isOutput=False)
    lnb_d = dp("lnb", [2, H], F32, isOutput=False)
    id8_d = dp("id8", [8, 8], F32, isOutput=False)
    w1_d = [dp(f"w1_{h}", [128, 8 * 128], F32, isOutput=False) for h in range(4)]
    b1_d = [dp(f"b1_{h}", [128, 2], F32, isOutput=False) for h in range(4)]
    mh_d = [dp(f"mh_{h}", [128, 256], F32, isOutput=False) for h in range(4)]
    fw1_d = dp("fw1", [128, 8 * 256], F32, isOutput=False)
    fb1_d = dp("fb1", [128, 4], F32, isOutput=False)
    fw2_d = dp("fw2", [128, 10], F32, isOutput=False)
    fb2_d = dp("fb2", [5, 1], F32, isOutput=False)
    out_d = dp("out", [5, BPC], F32, isOutput=True)

    with tile.TileContext(nc) as tc, ExitStack() as ctx:
        const = ctx.enter_context(tc.tile_pool(name="const", bufs=1))
        hidp = ctx.enter_context(tc.tile_pool(name="hidp", bufs=BPC * NT))
        work = ctx.enter_context(tc.tile_pool(name="work", bufs=1))
        psmain = ctx.enter_context(tc.tile_pool(name="psmain", bufs=1, space="PSUM"))
        pssm = ctx.enter_context(tc.tile_pool(name="pssm", bufs=1, space="PSUM"))

        def cload(src, shape, nm):
            t = const.tile(list(shape), F32, name=f"c_{nm}", tag=f"c_{nm}")
            nc.gpsimd.dma_start(out=t[:], in_=src[:])
            return t

        wm_sb = cload(wm_d, (128, NK * 8), "wm")
        invc_sb = cload(invc_d, (8, 1), "invc")
        lng_sb = cload(lng_d, (2, H), "lng")
        lnb_sb = cload(lnb_d, (2, H), "lnb")
        id8_sb = cload(id8_d, (8, 8), "id8")
        w1_sb = [cload(w1_d[h], (128, 8 * 128), f"w1{h}") for h in range(4)]
        b1_sb = [cload(b1_d[h], (128, 2), f"b1{h}") for h in range(4)]
        mh_sb = [cload(mh_d[h], (128, 256), f"mh{h}") for h in range(4)]
        fw1_sb = cload(fw1_d, (128, 8 * 256), "fw1")
        fb1_sb = cload(fb1_d, (128, 4), "fb1")
        fw2_sb = cload(fw2_d, (128, 10), "fw2")
        fb2_sb = cload(fb2_d, (5, 1), "fb2")

        # Wait-absorbers: a Matmult (LDWEIGHTS) can carry only ONE semaphore
        # wait, so each PE-read constant is consumed by a dummy matmul first;
        # the real matmuls then only wait on their streaming data input.
        scr_ps = pssm.tile([8, 8], F32)

        def absorb(csb, k=8):
            return nc.tensor.matmul(
                scr_ps[:, :], lhsT=csb[:, 0:k], rhs=csb[:, 0:k],
                start=True, stop=True,
            )

        wm_abs = absorb(wm_sb)

        # ---- main loop: pooled4[j, h] = sum_s wm[s, j] * hidden[s, h] ----
        pooled_ps = psmain.tile([8, H], F32)
        first_mm = None
        last_mm = None
        for b in range(BPC):
            for t in range(NT):
                ht = hidp.tile([128, TS // 128, H], F32)
                nc.sync.dma_start(
                    out=ht[:],
                    in_=hid_d[b, t * TS:(t + 1) * TS, :].rearrange(
                        "(c p) h -> p c h", p=128
                    ),
                )
                for c in range(TS // 128):
                    n = b * (S // 128) + t * (TS // 128) + c
                    lw = wm_sb[:, n * 8:(n + 1) * 8]
                    for j in range(2):
                        mm = nc.tensor.matmul(
                            pooled_ps[:, j * 512:(j + 1) * 512],
                            lhsT=lw,
                            rhs=ht[:, c, j * 512:(j + 1) * 512],
                            start=(n == 0),
                            stop=(n == NK - 1),
                        )
                        if first_mm is None:
                            first_mm = mm
                        last_mm = mm

        add_dep_helper(first_mm.ins, wm_abs.ins, sync=False, reason="absorb wm dma wait")

        # absorbers for epilogue PE constants; anchored after the main loop
        # (PE is idle during the DVE epilogue prologue) and before the first
        # epilogue matmul (the transpose chain).
        epi_abs = [absorb(c) for c in [id8_sb] + w1_sb + mh_sb + [fw1_sb, fw2_sb]]
        for a in epi_abs:
            add_dep_helper(a.ins, last_mm.ins, sync=False, reason="absorber after main loop")

        # Const touches on DVE/ACT: like the PE absorbers, every engine
        # instruction carries at most one semaphore wait, so consume each
        # const's DMA completion on the engine that will read it.
        def vtouch(c, nm):
            s = work.tile([1, 1], F32, name=f"tv_{nm}", tag=f"tv_{nm}")
            return nc.vector.tensor_copy(s[0:1, 0:1], c[0:1, 0:1])

        def atouch(c, nm):
            s = work.tile([128, 1], F32, name=f"ta_{nm}", tag=f"ta_{nm}")
            return nc.scalar.copy(out=s[:, 0:1], in_=c[:, 0:1])

        t_invc = vtouch(invc_sb, "invc")
        t_lng = vtouch(lng_sb, "lng")
        t_lnb = vtouch(lnb_sb, "lnb")
        t_fb2 = vtouch(fb2_sb, "fb2")
        a_b1 = [atouch(b1_sb[h], f"b1{h}") for h in range(4)]
        a_fb1 = atouch(fb1_sb, "fb1")

        # ---- epilogue ----
        # Compute-engine APs must start at partition 0/32/64/96, so all
        # cross-row arithmetic happens after transposing to the free dim.
        # P4 rows: 0-1 pooled(s0,s1), 2-3 first, 4-5 second, 6-7 ending
        P4 = work.tile([8, H], F32)
        p4op = nc.vector.tensor_scalar_mul(out=P4[:], in0=pooled_ps[:], scalar1=invc_sb[:])
        add_dep_helper(p4op.ins, t_invc.ins, sync=False, reason="invc touch first")

        # LayerNorm on pooled rows [0:2] -> THR
        stats = work.tile([2, 2, 6], F32)
        nc.vector.bn_stats(out=stats[:, 0, :], in_=P4[0:2, 0:512])
        nc.vector.bn_stats(out=stats[:, 1, :], in_=P4[0:2, 512:1024])
        mv = work.tile([2, 2], F32)
        nc.vector.bn_aggr(out=mv[:], in_=stats[:])
        eps_sb = work.tile([2, 1], F32)
        nc.vector.memset(eps_sb[:], LN_EPS)
        rstd = work.tile([2, 1], F32)
        nc.scalar.activation(
            out=rstd[:], in_=mv[:, 1:2],
            func=mybir.ActivationFunctionType.Sqrt, bias=eps_sb[:], scale=1.0,
        )
        nc.vector.reciprocal(rstd[:], rstd[:])
        xn = work.tile([2, H], F32)
        nc.vector.tensor_scalar(
            out=xn[:], in0=P4[0:2, :], scalar1=mv[:, 0:1], scalar2=rstd[:],
            op0=mybir.AluOpType.subtract, op1=mybir.AluOpType.mult,
        )
        THR = work.tile([2, H], F32)
        thr_mul = nc.vector.tensor_mul(THR[:], xn[:], lng_sb[:])
        thr_add = nc.vector.tensor_add(THR[:], THR[:], lnb_sb[:])
        add_dep_helper(thr_mul.ins, t_lng.ins, sync=False, reason="lng touch first")
        add_dep_helper(thr_add.ins, t_lnb.ins, sync=False, reason="lnb touch first")

        # XTR[:, 10c + r]: r in 0..8 = P4 row r, r in 8..10 = THR row r-8,
        # for H positions c*128..(c+1)*128 on partitions.
        xtr_ps = pssm.tile([128, 80], F32)
        first_tr = None
        for cc in range(8):
            tr = nc.tensor.transpose(
                out=xtr_ps[:, cc * 10:cc * 10 + 8],
                in_=P4[:, cc * 128:(cc + 1) * 128],
                identity=id8_sb[:],
            )
            if first_tr is None:
                first_tr = tr
                for a in epi_abs:
                    add_dep_helper(first_tr.ins, a.ins, sync=False, reason="absorbers before epilogue")
            nc.tensor.transpose(
                out=xtr_ps[:, cc * 10 + 8:cc * 10 + 10],
                in_=THR[:, cc * 128:(cc + 1) * 128],
                identity=id8_sb[0:2, 0:2],
            )
        XTR = work.tile([128, 8, 10], F32)
        nc.vector.tensor_copy(XTR[:], xtr_ps[:].rearrange("p (c r) -> p c r", r=10))

        # head inputs on the free dim: esc = relu(second-first), res = relu(-d)
        dT = work.tile([128, 8, 2], F32)
        nc.vector.tensor_sub(dT[:], XTR[:, :, 4:6], XTR[:, :, 2:4])
        escT = work.tile([128, 8, 2], F32)
        nc.vector.tensor_scalar_max(out=escT[:], in0=dT[:], scalar1=0.0)
        resT = work.tile([128, 8, 2], F32)
        nc.vector.tensor_scalar(
            out=resT[:], in0=dT[:], scalar1=-1.0, scalar2=0.0,
            op0=mybir.AluOpType.mult, op1=mybir.AluOpType.max,
        )

        def head_rhs(h, cc):
            if h == 0:
                return escT[:, cc, :]
            if h == 1:
                return resT[:, cc, :]
            if h == 2:
                return XTR[:, cc, 6:8]
            return XTR[:, cc, 8:10]

        # head first layers: h1[:, 2h+j] = w1_h.T @ x_{h,j}
        h1_ps = pssm.tile([128, 8], F32)
        for h in range(4):
            for cc in range(8):
                nc.tensor.matmul(
                    h1_ps[:, 2 * h:2 * h + 2],
                    lhsT=w1_sb[h][:, cc * 128:(cc + 1) * 128],
                    rhs=head_rhs(h, cc),
                    start=(cc == 0),
                    stop=(cc == 7),
                )
        # exact GELU: z=x+b1, y=erf(z/sqrt(2)), g=z+z*y  (0.5 folded into mh)
        z1 = work.tile([128, 8], F32)
        y1 = work.tile([128, 8], F32)
        for h in range(4):
            sl = slice(2 * h, 2 * h + 2)
            y1op = nc.scalar.activation(
                out=y1[:, sl], in_=h1_ps[:, sl],
                func=mybir.ActivationFunctionType.Erf,
                bias=b1_sb[h][:, 1:2], scale=RS2,
            )
            add_dep_helper(y1op.ins, a_b1[h].ins, sync=False, reason="b1 touch first")
            nc.scalar.activation(
                out=z1[:, sl], in_=h1_ps[:, sl],
                func=mybir.ActivationFunctionType.Identity,
                bias=b1_sb[h][:, 0:1], scale=1.0,
            )
        t1 = work.tile([128, 8], F32)
        nc.vector.tensor_mul(t1[:], z1[:], y1[:])
        g1 = work.tile([128, 8], F32)
        nc.vector.tensor_add(g1[:], z1[:], t1[:])

        # fc1[:, 2m+j] = fc_w1[:1024,mslice].T @ pooled_j + sum_h mh_h[mslice].T @ g1_{h,j}
        fc1_ps = pssm.tile([128, 4], F32)
        for m in range(2):
            sl = slice(2 * m, 2 * m + 2)
            for cc in range(8):
                nc.tensor.matmul(
                    fc1_ps[:, sl],
                    lhsT=fw1_sb[:, cc * 256 + m * 128:cc * 256 + (m + 1) * 128],
                    rhs=XTR[:, cc, 8:10],
                    start=(cc == 0),
                    stop=False,
                )
            for h in range(4):
                nc.tensor.matmul(
                    fc1_ps[:, sl],
                    lhsT=mh_sb[h][:, m * 128:(m + 1) * 128],
                    rhs=g1[:, 2 * h:2 * h + 2],
                    start=False,
                    stop=(h == 3),
                )
        z2 = work.tile([128, 4], F32)
        y2 = work.tile([128, 4], F32)
        for m in range(2):
            sl = slice(2 * m, 2 * m + 2)
            y2op = nc.scalar.activation(
                out=y2[:, sl], in_=fc1_ps[:, sl],
                func=mybir.ActivationFunctionType.Erf,
                bias=fb1_sb[:, 2 + m:3 + m], scale=RS2,
            )
            if m == 0:
                add_dep_helper(y2op.ins, a_fb1.ins, sync=False, reason="fb1 touch first")
            nc.scalar.activation(
                out=z2[:, sl], in_=fc1_ps[:, sl],
                func=mybir.ActivationFunctionType.Identity,
                bias=fb1_sb[:, m:m + 1], scale=1.0,
            )
        t2 = work.tile([128, 4], F32)
        nc.vector.tensor_mul(t2[:], z2[:], y2[:])
        g2 = work.tile([128, 4], F32)
        nc.vector.tensor_add(g2[:], z2[:], t2[:])

        out_ps = pssm.tile([5, 2], F32)
        for m in range(2):
            nc.tensor.matmul(
                out_ps[:],
                lhsT=fw2_sb[:, 5 * m:5 * m + 5],
                rhs=g2[:, 2 * m:2 * m + 2],
                start=(m == 0),
                stop=(m == 1),
            )
        out_sb = work.tile([5, 2], F32)
        oadd = nc.vector.tensor_scalar_add(out=out_sb[:], in0=out_ps[:], scalar1=fb2_sb[:])
        add_dep_helper(oadd.ins, t_fb2.ins, sync=False, reason="fb2 touch first")
        nc.sync.dma_start(out=out_d[:, :], in_=out_sb[:])

    nc.compile()
    return nc


def _pack_k_major(w, k, m):
    """[K, M] -> [128, (K//128)*M] with lhsT chunk c at cols [c*M, (c+1)*M)."""
    return np.ascontiguousarray(
        w.reshape(k // 128, 128, m).transpose(1, 0, 2).reshape(128, (k // 128) * m)
    ).astype(np.float32)


def _host_prep(inputs):
    """Build all per-core in_maps from the full inputs."""
    f32 = np.float32
    am = np.asarray(inputs["attention_mask"])
    hid = np.asarray(inputs["hidden"], dtype=f32)

    m_full = am.astype(f32)                      # [B, S]
    L = am.astype(np.int64).sum(1)               # [B]
    pos = np.arange(S)[None, :]
    mid = (L // 2)[:, None]
    Lb = L[:, None]
    st = np.maximum(1, L - 64)[:, None]
    fm = ((pos >= 1) & (pos < mid)).astype(f32)
    sm = ((pos >= mid) & (pos < Lb - 1)).astype(f32)
    em = ((pos >= st) & (pos < Lb - 1)).astype(f32)
    masks = [m_full, fm, sm, em]                 # type order: pooled,first,second,ending
    invs = [
        (1.0 / np.maximum(mk.sum(1, dtype=np.float64), EPS)).astype(f32)
        for mk in masks
    ]

    ln_g = np.asarray(inputs["ln_g"], f32)
    ln_b = np.asarray(inputs["ln_b"], f32)
    lng = np.ascontiguousarray(np.broadcast_to(ln_g, (2, H))).astype(f32)
    lnb = np.ascontiguousarray(np.broadcast_to(ln_b, (2, H))).astype(f32)
    id8 = np.eye(8, dtype=f32)

    fc_w1 = np.asarray(inputs["fc_w1"], f32)     # [H+4, 256]
    fc_b1 = np.asarray(inputs["fc_b1"], f32)
    fc_w2 = np.asarray(inputs["fc_w2"], f32)     # [256, 5]
    fc_b2 = np.asarray(inputs["fc_b2"], f32)

    w1p, b1p, mhp = [], [], []
    fb1_eff = fc_b1.astype(np.float64).copy()
    for h, name in enumerate(HEADS):
        w1 = np.asarray(inputs[f"{name}_w1"], f32)   # [H, 128]
        b1 = np.asarray(inputs[f"{name}_b1"], f32)   # [128]
        w2 = np.asarray(inputs[f"{name}_w2"], f32)   # [128, 1]
        b2 = np.asarray(inputs[f"{name}_b2"], f32)   # [1]
        w1p.append(_pack_k_major(w1, H, 128))
        b1p.append(np.stack([b1, b1 * RS2], axis=1).astype(f32))
        mhp.append(
            np.ascontiguousarray(0.5 * w2[:, 0][:, None] * fc_w1[H + h, :][None, :]).astype(f32)
        )
        fb1_eff += b2[0] * fc_w1[H + h, :].astype(np.float64)
    fb1_eff = fb1_eff.astype(f32)

    fw1 = _pack_k_major(fc_w1[:H], H, 256)
    fb1 = np.stack(
        [fb1_eff[0:128], fb1_eff[128:256], fb1_eff[0:128] * RS2, fb1_eff[128:256] * RS2],
        axis=1,
    ).astype(f32)
    fw2 = _pack_k_major(0.5 * fc_w2, 256, 5)
    fb2 = np.asarray(fc_b2, f32)[:, None]

    shared = dict(lng=lng, lnb=lnb, id8=id8, fw1=fw1, fb1=fb1, fw2=fw2, fb2=fb2)
    for h in range(4):
        shared[f"w1_{h}"] = w1p[h]
        shared[f"b1_{h}"] = b1p[h]
        shared[f"mh_{h}"] = mhp[h]

    in_maps = []
    for i in range(NCORES):
        msk = np.zeros((BPC, S // 128, 128, 8), f32)
        invc = np.zeros((8, 1), f32)
        for b in range(BPC):
            gb = BPC * i + b
            for ty in range(4):
                msk[b, :, :, 2 * ty + b] = masks[ty][gb].reshape(S // 128, 128)
                invc[2 * ty + b, 0] = invs[ty][gb]
        wm = np.ascontiguousarray(
            msk.reshape(NK, 128, 8).transpose(1, 0, 2).reshape(128, NK * 8)
        )
        in_maps.append(
            dict(
                hid=np.ascontiguousarray(hid[BPC * i:BPC * (i + 1)]),
                wm=wm,
                invc=invc,
                **shared,
            )
        )
    return in_maps


def kernel(**inputs):
    if "nc" not in _NC_CACHE:
        _NC_CACHE["nc"] = _build_nc()
    nc = _NC_CACHE["nc"]
    in_maps = _host_prep(inputs)
    res = run_bass_kernel_spmd(nc, in_maps, core_ids=list(range(NCORES)))
    out = np.empty((B, 5), np.float32)
    for i in range(NCORES):
        out[BPC * i:BPC * (i + 1)] = res.results[i]["out"].T
    return out


# revision 27
# speedup vs baseline: 1.8350x; 1.8350x over previous
"""Trainium2 Bass kernel for nn_DirectionalMultiHeadClassifier.

Data-parallel over 8 NeuronCores: each core handles 2 of the 16 samples.

Math per sample (mirrors the reference):
  - 4 masked means over S of hidden [S,H]: full attention_mask, and three
    position-range masks derived from L = mask.sum() (first/second/ending).
    Computed on-device as one PSUM-accumulated matmul:
        pooled4[8, H] += W_chunk[128, 8].T @ hidden_chunk[128, H]
    where W is a host-built 0/1 mask matrix (4 mask types x 2 samples) and
    the 1/count normalization is applied afterwards.
  - LayerNorm on the full-mask pooled vector.
  - 4 small MLP heads (H->128 -> exact GELU -> 128->1). The scalar head
    outputs only feed the final classifier's last 4 input features, so the
    128->1 layer is folded into the classifier on the host:
        fc1 += gelu_h @ (0.5 * w2_h outer fc_w1[1024+h, :])
        fc_b1_eff = fc_b1 + sum_h b2_h * fc_w1[1024+h, :]
  - Final classifier (1028->256 -> exact GELU -> 256->5).
  Exact GELU is computed as 0.5*z*(1+erf(z/sqrt(2))) with the 0.5 folded
  into the following layer's weights.
"""

import ml_dtypes
import numpy as np

import concourse.bass as bass
import concourse.tile as tile
from bass_rust import add_dep_helper
from concourse import bacc, mybir
from concourse.bass_utils import run_bass_kernel_spmd

B, S, H = 16, 2048, 1024
NCORES = 8
BPC = B // NCORES          # samples per core
NK = BPC * (S // 128)      # 128-row contraction chunks per core
TS = 1024                  # S rows per hidden DMA tile (4 MiB)
NT = S // TS               # DMA tiles per sample
RS2 = 0.7071067811865476   # 1/sqrt(2)
LN_EPS = 1e-5
EPS = 1e-9
F32 = mybir.dt.float32
BF16 = mybir.dt.bfloat16
HEADS = ["esc", "res", "end", "thr"]

_NC_CACHE = {}


def _build_nc():
    """Build the per-core Bass program (identical on all 8 cores)."""
    from contextlib import ExitStack

    nc = bacc.Bacc(
        "TRN2", target_bir_lowering=False, debug=False, num_devices=NCORES
    )
    dp = nc.declare_dram_parameter
    hid_d = dp("hid", [BPC, S, H], BF16, isOutput=False)
    wm_d = dp("wm", [128, NK * 8], BF16, isOutput=False)
    invc_d = dp("invc", [8, 1], F32, isOutput=False)
    lng_d = dp("lng", [2, H], F32, isOutput=False)
    lnb_d = dp("lnb", [2, H], F32, isOutput=False)
    id8_d = dp("id8", [8, 8], F32, isOutput=False)
    w1_d = [dp(f"w1_{h}", [128, 8 * 128], BF16, isOutput=False) for h in range(4)]
    b1_d = [dp(f"b1_{h}", [128, 2], F32, isOutput=False) for h in range(4)]
    mh_d = [dp(f"mh_{h}", [128, 256], BF16, isOutput=False) for h in range(4)]
    fw1_d = dp("fw1", [128, 8 * 256], BF16, isOutput=False)
    fb1_d = dp("fb1", [128, 4], F32, isOutput=False)
    fw2_d = dp("fw2", [128, 10], BF16, isOutput=False)
    fb2_d = dp("fb2", [5, 1], F32, isOutput=False)
    out_d = dp("out", [5, BPC], F32, isOutput=True)

    with tile.TileContext(nc) as tc, ExitStack() as ctx:
        const = ctx.enter_context(tc.tile_pool(name="const", bufs=1))
        hidp = ctx.enter_context(tc.tile_pool(name="hidp", bufs=BPC * NT))
        work = ctx.enter_context(tc.tile_pool(name="work", bufs=1))
        psmain = ctx.enter_context(tc.tile_pool(name="psmain", bufs=1, space="PSUM"))
        pssm = ctx.enter_context(tc.tile_pool(name="pssm", bufs=1, space="PSUM"))

        def cload(src, shape, nm, dt=F32):
            t = const.tile(list(shape), dt, name=f"c_{nm}", tag=f"c_{nm}")
            nc.gpsimd.dma_start(out=t[:], in_=src[:])
            return t

        wm_sb = cload(wm_d, (128, NK * 8), "wm", BF16)
        invc_sb = cload(invc_d, (8, 1), "invc")
        lng_sb = cload(lng_d, (2, H), "lng")
        lnb_sb = cload(lnb_d, (2, H), "lnb")
        id8_sb = cload(id8_d, (8, 8), "id8")
        w1_sb = [cload(w1_d[h], (128, 8 * 128), f"w1{h}", BF16) for h in range(4)]
        b1_sb = [cload(b1_d[h], (128, 2), f"b1{h}") for h in range(4)]
        mh_sb = [cload(mh_d[h], (128, 256), f"mh{h}", BF16) for h in range(4)]
        fw1_sb = cload(fw1_d, (128, 8 * 256), "fw1", BF16)
        fb1_sb = cload(fb1_d, (128, 4), "fb1")
        fw2_sb = cload(fw2_d, (128, 10), "fw2", BF16)
        fb2_sb = cload(fb2_d, (5, 1), "fb2")

        # Wait-absorbers: a Matmult (LDWEIGHTS) can carry only ONE semaphore
        # wait, so each PE-read constant is consumed by a dummy matmul first;
        # the real matmuls then only wait on their streaming data input.
        scr_ps = pssm.tile([8, 8], F32)

        def absorb(csb, k=8):
            return nc.tensor.matmul(
                scr_ps[:, :], lhsT=csb[:, 0:k], rhs=csb[:, 0:k],
                start=True, stop=True,
            )

        wm_abs = absorb(wm_sb)

        # ---- main loop: pooled4[j, h] = sum_s wm[s, j] * hidden[s, h] ----
        pooled_ps = psmain.tile([8, H], F32)
        first_mm = None
        last_mm = None
        for b in range(BPC):
            for t in range(NT):
                ht = hidp.tile([128, TS // 128, H], BF16)
                # alternate the two HWDGE rings (sync / scalar) so transfers
                # overlap instead of serializing on one sequencer ring
                dma_eng = nc.sync if (b * NT + t) % 2 == 0 else nc.scalar
                dma_eng.dma_start(
                    out=ht[:],
                    in_=hid_d[b, t * TS:(t + 1) * TS, :].rearrange(
                        "(c p) h -> p c h", p=128
                    ),
                )
                for c in range(TS // 128):
                    n = b * (S // 128) + t * (TS // 128) + c
                    lw = wm_sb[:, n * 8:(n + 1) * 8]
                    for j in range(2):
                        mm = nc.tensor.matmul(
                            pooled_ps[:, j * 512:(j + 1) * 512],
                            lhsT=lw,
                            rhs=ht[:, c, j * 512:(j + 1) * 512],
                            start=(n == 0),
                            stop=(n == NK - 1),
                        )
                        if first_mm is None:
                            first_mm = mm
                        last_mm = mm

        add_dep_helper(first_mm.ins, wm_abs.ins, sync=False, reason="absorb wm dma wait")

        # absorbers for epilogue PE constants; anchored after the main loop
        # (PE is idle during the DVE epilogue prologue) and before the first
        # epilogue matmul (the transpose chain).
        epi_abs = [absorb(c) for c in [id8_sb] + w1_sb + mh_sb + [fw1_sb, fw2_sb]]
        for a in epi_abs:
            add_dep_helper(a.ins, last_mm.ins, sync=False, reason="absorber after main loop")

        # Const touches on DVE/ACT: like the PE absorbers, every engine
        # instruction carries at most one semaphore wait, so consume each
        # const's DMA completion on the engine that will read it.
        def vtouch(c, nm):
            s = work.tile([1, 1], F32, name=f"tv_{nm}", tag=f"tv_{nm}")
            return nc.vector.tensor_copy(s[0:1, 0:1], c[0:1, 0:1])

        def atouch(c, nm):
            s = work.tile([128, 1], F32, name=f"ta_{nm}", tag=f"ta_{nm}")
            return nc.scalar.copy(out=s[:, 0:1], in_=c[:, 0:1])

        t_invc = vtouch(invc_sb, "invc")
        t_lng = vtouch(lng_sb, "lng")
        t_lnb = vtouch(lnb_sb, "lnb")
        t_fb2 = vtouch(fb2_sb, "fb2")
        a_b1 = [atouch(b1_sb[h], f"b1{h}") for h in range(4)]
        a_fb1 = atouch(fb1_sb, "fb1")

        # ---- epilogue ----
        # Compute-engine APs must start at partition 0/32/64/96, so all
        # cross-row arithmetic happens after transposing to the free dim.
        # P4 rows: 0-1 pooled(s0,s1), 2-3 first, 4-5 second, 6-7 ending
        P4 = work.tile([8, H], F32)
        p4op = nc.vector.tensor_scalar_mul(out=P4[:], in0=pooled_ps[:], scalar1=invc_sb[:])
        add_dep_helper(p4op.ins, t_invc.ins, sync=False, reason="invc touch first")

        # LayerNorm on pooled rows [0:2] -> THR
        stats = work.tile([2, 2, 6], F32)
        nc.vector.bn_stats(out=stats[:, 0, :], in_=P4[0:2, 0:512])
        nc.vector.bn_stats(out=stats[:, 1, :], in_=P4[0:2, 512:1024])
        mv = work.tile([2, 2], F32)
        nc.vector.bn_aggr(out=mv[:], in_=stats[:])
        eps_sb = work.tile([2, 1], F32)
        nc.vector.memset(eps_sb[:], LN_EPS)
        rstd = work.tile([2, 1], F32)
        nc.scalar.activation(
            out=rstd[:], in_=mv[:, 1:2],
            func=mybir.ActivationFunctionType.Sqrt, bias=eps_sb[:], scale=1.0,
        )
        nc.vector.reciprocal(rstd[:], rstd[:])
        xn = work.tile([2, H], F32)
        nc.vector.tensor_scalar(
            out=xn[:], in0=P4[0:2, :], scalar1=mv[:, 0:1], scalar2=rstd[:],
            op0=mybir.AluOpType.subtract, op1=mybir.AluOpType.mult,
        )
        THR = work.tile([2, H], F32)
        thr_mul = nc.vector.tensor_mul(THR[:], xn[:], lng_sb[:])
        thr_add = nc.vector.tensor_add(THR[:], THR[:], lnb_sb[:])
        add_dep_helper(thr_mul.ins, t_lng.ins, sync=False, reason="lng touch first")
        add_dep_helper(thr_add.ins, t_lnb.ins, sync=False, reason="lnb touch first")

        # XTR[:, 10c + r]: r in 0..8 = P4 row r, r in 8..10 = THR row r-8,
        # for H positions c*128..(c+1)*128 on partitions.
        xtr_ps = pssm.tile([128, 80], F32)
        first_tr = None
        for cc in range(8):
            tr = nc.tensor.transpose(
                out=xtr_ps[:, cc * 10:cc * 10 + 8],
                in_=P4[:, cc * 128:(cc + 1) * 128],
                identity=id8_sb[:],
            )
            if first_tr is None:
                first_tr = tr
                for a in epi_abs:
                    add_dep_helper(first_tr.ins, a.ins, sync=False, reason="absorbers before epilogue")
            nc.tensor.transpose(
                out=xtr_ps[:, cc * 10 + 8:cc * 10 + 10],
                in_=THR[:, cc * 128:(cc + 1) * 128],
                identity=id8_sb[0:2, 0:2],
            )
        XTR = work.tile([128, 8, 10], BF16)
        nc.vector.tensor_copy(XTR[:], xtr_ps[:].rearrange("p (c r) -> p c r", r=10))

        # head inputs on the free dim: esc = relu(second-first), res = relu(-d)
        dT = work.tile([128, 8, 2], BF16)
        nc.vector.tensor_sub(dT[:], XTR[:, :, 4:6], XTR[:, :, 2:4])
        escT = work.tile([128, 8, 2], BF16)
        nc.vector.tensor_scalar_max(out=escT[:], in0=dT[:], scalar1=0.0)
        resT = work.tile([128, 8, 2], BF16)
        nc.vector.tensor_scalar(
            out=resT[:], in0=dT[:], scalar1=-1.0, scalar2=0.0,
            op0=mybir.AluOpType.mult, op1=mybir.AluOpType.max,
        )

        def head_rhs(h, cc):
            if h == 0:
                return escT[:, cc, :]
            if h == 1:
                return resT[:, cc, :]
            if h == 2:
                return XTR[:, cc, 6:8]
            return XTR[:, cc, 8:10]

        # head first layers: h1[:, 2h+j] = w1_h.T @ x_{h,j}
        h1_ps = pssm.tile([128, 8], F32)
        for h in range(4):
            for cc in range(8):
                nc.tensor.matmul(
                    h1_ps[:, 2 * h:2 * h + 2],
                    lhsT=w1_sb[h][:, cc * 128:(cc + 1) * 128],
                    rhs=head_rhs(h, cc),
                    start=(cc == 0),
                    stop=(cc == 7),
                )
        # exact GELU: z=x+b1, y=erf(z/sqrt(2)), g=z+z*y  (0.5 folded into mh)
        z1 = work.tile([128, 8], F32)
        y1 = work.tile([128, 8], F32)
        for h in range(4):
            sl = slice(2 * h, 2 * h + 2)
            y1op = nc.scalar.activation(
                out=y1[:, sl], in_=h1_ps[:, sl],
                func=mybir.ActivationFunctionType.Erf,
                bias=b1_sb[h][:, 1:2], scale=RS2,
            )
            add_dep_helper(y1op.ins, a_b1[h].ins, sync=False, reason="b1 touch first")
            nc.scalar.activation(
                out=z1[:, sl], in_=h1_ps[:, sl],
                func=mybir.ActivationFunctionType.Identity,
                bias=b1_sb[h][:, 0:1], scale=1.0,
            )
        t1 = work.tile([128, 8], F32)
        nc.vector.tensor_mul(t1[:], z1[:], y1[:])
        g1 = work.tile([128, 8], BF16)
        nc.vector.tensor_add(g1[:], z1[:], t1[:])

        # fc1[:, 2m+j] = fc_w1[:1024,mslice].T @ pooled_j + sum_h mh_h[mslice].T @ g1_{h,j}
        fc1_ps = pssm.tile([128, 4], F32)
        for m in range(2):
            sl = slice(2 * m, 2 * m + 2)
            for cc in range(8):
                nc.tensor.matmul(
                    fc1_ps[:, sl],
                    lhsT=fw1_sb[:, cc * 256 + m * 128:cc * 256 + (m + 1) * 128],
                    rhs=XTR[:, cc, 8:10],
                    start=(cc == 0),
                    stop=False,
                )
            for h in range(4):
                nc.tensor.matmul(
                    fc1_ps[:, sl],
                    lhsT=mh_sb[h][:, m * 128:(m + 1) * 128],
                    rhs=g1[:, 2 * h:2 * h + 2],
                    start=False,
                    stop=(h == 3),
                )
        z2 = work.tile([128, 4], F32)
        y2 = work.tile([128, 4], F32)
        for m in range(2):
            sl = slice(2 * m, 2 * m + 2)
            y2op = nc.scalar.activation(
                out=y2[:, sl], in_=fc1_ps[:, sl],
                func=mybir.ActivationFunctionType.Erf,
                bias=fb1_sb[:, 2 + m:3 + m], scale=RS2,
            )
            if m == 0:
                add_dep_helper(y2op.ins, a_fb1.ins, sync=False, reason="fb1 touch first")
            nc.scalar.activation(
                out=z2[:, sl], in_=fc1_ps[:, sl],
                func=mybir.ActivationFunctionType.Identity,
                bias=fb1_sb[:, m:m + 1], scale=1.0,
            )
        t2 = work.tile([128, 4], F32)
        nc.vector.tensor_mul(t2[:], z2[:], y2[:])
        g2 = work.tile([128, 4], BF16)
        nc.vector.tensor_add(g2[:], z2[:], t2[:])

        out_ps = pssm.tile([5, 2], F32)
        for m in range(2):
            nc.tensor.matmul(
                out_ps[:],
                lhsT=fw2_sb[:, 5 * m:5 * m + 5],
                rhs=g2[:, 2 * m:2 * m + 2],
                start=(m == 0),
                stop=(m == 1),
            )
        out_sb = work.tile([5, 2], F32)
        oadd = nc.vector.tensor_scalar_add(out=out_sb[:], in0=out_ps[:], scalar1=fb2_sb[:])
        add_dep_helper(oadd.ins, t_fb2.ins, sync=False, reason="fb2 touch first")
        nc.sync.dma_start(out=out_d[:, :], in_=out_sb[:])

    nc.compile()
    return nc


def _pack_k_major(w, k, m):
    """[K, M] -> [128, (K//128)*M] with lhsT chunk c at cols [c*M, (c+1)*M)."""
    return np.ascontiguousarray(
        w.reshape(k // 128, 128, m).transpose(1, 0, 2).reshape(128, (k // 128) * m)
    ).astype(np.float32)


def _host_prep(inputs):
    """Build all per-core in_maps from the full inputs."""
    f32 = np.float32
    am = np.asarray(inputs["attention_mask"])
    hid = np.asarray(inputs["hidden"], dtype=f32)

    m_full = am.astype(f32)                      # [B, S]
    L = am.astype(np.int64).sum(1)               # [B]
    pos = np.arange(S)[None, :]
    mid = (L // 2)[:, None]
    Lb = L[:, None]
    st = np.maximum(1, L - 64)[:, None]
    fm = ((pos >= 1) & (pos < mid)).astype(f32)
    sm = ((pos >= mid) & (pos < Lb - 1)).astype(f32)
    em = ((pos >= st) & (pos < Lb - 1)).astype(f32)
    masks = [m_full, fm, sm, em]                 # type order: pooled,first,second,ending
    invs = [
        (1.0 / np.maximum(mk.sum(1, dtype=np.float64), EPS)).astype(f32)
        for mk in masks
    ]

    ln_g = np.asarray(inputs["ln_g"], f32)
    ln_b = np.asarray(inputs["ln_b"], f32)
    lng = np.ascontiguousarray(np.broadcast_to(ln_g, (2, H))).astype(f32)
    lnb = np.ascontiguousarray(np.broadcast_to(ln_b, (2, H))).astype(f32)
    id8 = np.eye(8, dtype=f32)

    fc_w1 = np.asarray(inputs["fc_w1"], f32)     # [H+4, 256]
    fc_b1 = np.asarray(inputs["fc_b1"], f32)
    fc_w2 = np.asarray(inputs["fc_w2"], f32)     # [256, 5]
    fc_b2 = np.asarray(inputs["fc_b2"], f32)

    w1p, b1p, mhp = [], [], []
    fb1_eff = fc_b1.astype(np.float64).copy()
    for h, name in enumerate(HEADS):
        w1 = np.asarray(inputs[f"{name}_w1"], f32)   # [H, 128]
        b1 = np.asarray(inputs[f"{name}_b1"], f32)   # [128]
        w2 = np.asarray(inputs[f"{name}_w2"], f32)   # [128, 1]
        b2 = np.asarray(inputs[f"{name}_b2"], f32)   # [1]
        w1p.append(_pack_k_major(w1, H, 128).astype(ml_dtypes.bfloat16))
        b1p.append(np.stack([b1, b1 * RS2], axis=1).astype(f32))
        mhp.append(
            np.ascontiguousarray(
                0.5 * w2[:, 0][:, None] * fc_w1[H + h, :][None, :]
            ).astype(ml_dtypes.bfloat16)
        )
        fb1_eff += b2[0] * fc_w1[H + h, :].astype(np.float64)
    fb1_eff = fb1_eff.astype(f32)

    fw1 = _pack_k_major(fc_w1[:H], H, 256).astype(ml_dtypes.bfloat16)
    fb1 = np.stack(
        [fb1_eff[0:128], fb1_eff[128:256], fb1_eff[0:128] * RS2, fb1_eff[128:256] * RS2],
        axis=1,
    ).astype(f32)
    fw2 = _pack_k_major(0.5 * fc_w2, 256, 5).astype(ml_dtypes.bfloat16)
    fb2 = np.asarray(fc_b2, f32)[:, None]

    shared = dict(lng=lng, lnb=lnb, id8=id8, fw1=fw1, fb1=fb1, fw2=fw2, fb2=fb2)
    for h in range(4):
        shared[f"w1_{h}"] = w1p[h]
        shared[f"b1_{h}"] = b1p[h]
        shared[f"mh_{h}"] = mhp[h]

    in_maps = []
    for i in range(NCORES):
        msk = np.zeros((BPC, S // 128, 128, 8), f32)
        invc = np.zeros((8, 1), f32)
        for b in range(BPC):
            gb = BPC * i + b
            for ty in range(4):
                msk[b, :, :, 2 * ty + b] = masks[ty][gb].reshape(S // 128, 128)
                invc[2 * ty + b, 0] = invs[ty][gb]
        wm = np.ascontiguousarray(
            msk.reshape(NK, 128, 8).transpose(1, 0, 2).reshape(128, NK * 8)
        ).astype(ml_dtypes.bfloat16)
        in_maps.append(
            dict(
                hid=np.ascontiguousarray(hid[BPC * i:BPC * (i + 1)]).astype(
                    ml_dtypes.bfloat16
                ),
                wm=wm,
                invc=invc,
                **shared,
            )
        )
    return in_maps


def kernel(**inputs):
    if "nc" not in _NC_CACHE:
        _NC_CACHE["nc"] = _build_nc()
    nc = _NC_CACHE["nc"]
    in_maps = _host_prep(inputs)
    res = run_bass_kernel_spmd(nc, in_maps, core_ids=list(range(NCORES)))
    out = np.empty((B, 5), np.float32)
    for i in range(NCORES):
        out[BPC * i:BPC * (i + 1)] = res.results[i]["out"].T
    return out


# revision 28
# speedup vs baseline: 1.9639x; 1.0702x over previous
"""Trainium2 Bass kernel for nn_DirectionalMultiHeadClassifier.

Data-parallel over 8 NeuronCores: each core handles 2 of the 16 samples.

Math per sample (mirrors the reference):
  - 4 masked means over S of hidden [S,H]: full attention_mask, and three
    position-range masks derived from L = mask.sum() (first/second/ending).
    Computed on-device as one PSUM-accumulated matmul:
        pooled4[8, H] += W_chunk[128, 8].T @ hidden_chunk[128, H]
    where W is a host-built 0/1 mask matrix (4 mask types x 2 samples) and
    the 1/count normalization is applied afterwards.
  - LayerNorm on the full-mask pooled vector; ln_g/ln_b are folded on the
    host into every consumer of the normalized vector (thr head w1/b1 and
    the fc pooled-part weights/bias), so the device only normalizes.
  - 4 small MLP heads (H->128 -> exact GELU -> 128->1). The scalar head
    outputs only feed the final classifier's last 4 input features, so the
    128->1 layer is folded into the classifier on the host:
        fc1 += gelu_h @ (0.5 * w2_h outer fc_w1[1024+h, :])
        fc_b1_eff = fc_b1 + sum_h b2_h * fc_w1[1024+h, :]
  - Final classifier (1028->256 -> exact GELU -> 256->5).
  Exact GELU is computed as 0.5*z*(1+erf(z/sqrt(2))) with the 0.5 folded
  into the following layer's weights.

Compute dtype: hidden/masks/weights stream through the PE in bf16 (masks
are exact 0/1 in bf16); all accumulation is f32 in PSUM.
"""

import ml_dtypes
import numpy as np

import concourse.bass as bass
import concourse.tile as tile
from bass_rust import add_dep_helper
from concourse import bacc, mybir
from concourse.bass_utils import run_bass_kernel_spmd

B, S, H = 16, 2048, 1024
NCORES = 8
BPC = B // NCORES          # samples per core
NK = BPC * (S // 128)      # 128-row contraction chunks per core
TS = 1024                  # S rows per hidden DMA tile (2 MiB bf16)
NT = S // TS               # DMA tiles per sample
RS2 = 0.7071067811865476   # 1/sqrt(2)
LN_EPS = 1e-5
EPS = 1e-9
F32 = mybir.dt.float32
BF16 = mybir.dt.bfloat16
HEADS = ["esc", "res", "end", "thr"]

# packed bf16 const-block column offsets
CB_W1 = 0                  # 4 x [128, 1024]
CB_MH = 4096               # 4 x [128, 256]
CB_FW1 = 5120              # [128, 2048]
CB_FW2 = 7168              # [128, 10]
CB_COLS = 7178
# packed f32 const-block column offsets
CF_INVC = 0                # [8, 1]
CF_ID8 = 1                 # [8, 8]
CF_B1 = 9                  # 4 x [128, 2] (bias, bias/sqrt2)
CF_FB1 = 17                # [128, 4] (m0, m1, m0/sqrt2, m1/sqrt2)
CF_FB2 = 21                # [5, 1]
CF_COLS = 22

_NC_CACHE = {}


def _build_nc():
    """Build the per-core Bass program (identical on all 8 cores)."""
    from contextlib import ExitStack

    nc = bacc.Bacc(
        "TRN2", target_bir_lowering=False, debug=False, num_devices=NCORES
    )
    dp = nc.declare_dram_parameter
    hid_d = dp("hid", [BPC, S, H], BF16, isOutput=False)
    wm_d = dp("wm", [128, NK * 8], BF16, isOutput=False)
    cb_d = dp("cb", [128, CB_COLS], BF16, isOutput=False)
    cf_d = dp("cf", [128, CF_COLS], F32, isOutput=False)
    out_d = dp("out", [5, BPC], F32, isOutput=True)

    with tile.TileContext(nc) as tc, ExitStack() as ctx:
        const = ctx.enter_context(tc.tile_pool(name="const", bufs=1))
        hidp = ctx.enter_context(tc.tile_pool(name="hidp", bufs=BPC * NT))
        work = ctx.enter_context(tc.tile_pool(name="work", bufs=1))
        psmain = ctx.enter_context(tc.tile_pool(name="psmain", bufs=1, space="PSUM"))
        pssm = ctx.enter_context(tc.tile_pool(name="pssm", bufs=1, space="PSUM"))

        # ACT table warm-up: touch every activation function used later so
        # the ~1.3us/table loads overlap the initial DMAs instead of
        # serializing into the epilogue.
        ws_in = work.tile([1, 1], F32)
        ws_b = work.tile([1, 1], F32)
        ws_out = work.tile([1, 1], F32)
        nc.vector.memset(ws_in[:], 0.0)
        nc.vector.memset(ws_b[:], 0.0)
        for fn in (
            mybir.ActivationFunctionType.Erf,
            mybir.ActivationFunctionType.Sqrt,
            mybir.ActivationFunctionType.Identity,
        ):
            nc.scalar.activation(out=ws_out[:], in_=ws_in[:], func=fn, bias=ws_b[:])

        # wm first, on the fast HWDGE ring: the main loop's first matmul
        # needs only wm + the first hidden tile.
        wm_sb = const.tile([128, NK * 8], BF16, name="c_wm", tag="c_wm")
        nc.sync.dma_start(out=wm_sb[:], in_=wm_d[:])
        cb_sb = const.tile([128, CB_COLS], BF16, name="c_cb", tag="c_cb")
        nc.gpsimd.dma_start(out=cb_sb[:], in_=cb_d[:])
        cf_sb = const.tile([128, CF_COLS], F32, name="c_cf", tag="c_cf")
        nc.gpsimd.dma_start(out=cf_sb[:], in_=cf_d[:])

        # const views
        invc_v = cf_sb[0:8, CF_INVC:CF_INVC + 1]
        id8_v = cf_sb[0:8, CF_ID8:CF_ID8 + 8]
        i2_v = cf_sb[0:2, CF_ID8:CF_ID8 + 2]
        b1_v = lambda h: cf_sb[:, CF_B1 + 2 * h:CF_B1 + 2 * h + 1]
        b1s_v = lambda h: cf_sb[:, CF_B1 + 2 * h + 1:CF_B1 + 2 * h + 2]
        fb1_v = lambda m: cf_sb[:, CF_FB1 + m:CF_FB1 + m + 1]
        fb1s_v = lambda m: cf_sb[:, CF_FB1 + 2 + m:CF_FB1 + 3 + m]
        fb2_v = cf_sb[0:5, CF_FB2:CF_FB2 + 1]
        w1_v = lambda h, c: cb_sb[:, CB_W1 + 1024 * h + 128 * c:CB_W1 + 1024 * h + 128 * (c + 1)]
        mh_v = lambda h, m: cb_sb[:, CB_MH + 256 * h + 128 * m:CB_MH + 256 * h + 128 * (m + 1)]
        fw1_v = lambda c, m: cb_sb[:, CB_FW1 + 256 * c + 128 * m:CB_FW1 + 256 * c + 128 * (m + 1)]
        fw2_v = lambda m: cb_sb[:, CB_FW2 + 5 * m:CB_FW2 + 5 * (m + 1)]

        # Wait-absorbers: every engine instruction carries at most ONE
        # semaphore wait in this walrus build, so consume each const DMA's
        # completion once per reading engine; real consumers then only wait
        # on their data inputs.
        scr_ps = pssm.tile([8, 8], F32)

        def absorb(csb):
            return nc.tensor.matmul(
                scr_ps[:, :], lhsT=csb[:, 0:8], rhs=csb[:, 0:8],
                start=True, stop=True,
            )

        wm_abs = absorb(wm_sb)

        # ---- main loop: pooled4[j, h] = sum_s wm[s, j] * hidden[s, h] ----
        pooled_ps = psmain.tile([8, H], F32)
        first_mm = None
        last_mm = None
        for b in range(BPC):
            for t in range(NT):
                ht = hidp.tile([128, TS // 128, H], BF16)
                # alternate the two HWDGE rings (sync / scalar) so transfers
                # overlap instead of serializing on one sequencer ring
                dma_eng = nc.sync if (b * NT + t) % 2 == 0 else nc.scalar
                dma_eng.dma_start(
                    out=ht[:],
                    in_=hid_d[b, t * TS:(t + 1) * TS, :].rearrange(
                        "(c p) h -> p c h", p=128
                    ),
                )
                for c in range(TS // 128):
                    n = b * (S // 128) + t * (TS // 128) + c
                    lw = wm_sb[:, n * 8:(n + 1) * 8]
                    for j in range(2):
                        mm = nc.tensor.matmul(
                            pooled_ps[:, j * 512:(j + 1) * 512],
                            lhsT=lw,
                            rhs=ht[:, c, j * 512:(j + 1) * 512],
                            start=(n == 0),
                            stop=(n == NK - 1),
                        )
                        if first_mm is None:
                            first_mm = mm
                        last_mm = mm

        add_dep_helper(first_mm.ins, wm_abs.ins, sync=False, reason="absorb wm dma wait")

        # absorbers/touches for epilogue consts; PE ones anchored after the
        # main loop (PE is idle during the DVE epilogue prologue).
        cb_abs = absorb(cb_sb)
        cf_abs = absorb(cf_sb)
        for a in (cb_abs, cf_abs):
            add_dep_helper(a.ins, last_mm.ins, sync=False, reason="absorber after main loop")
        tv_cf = work.tile([1, 1], F32)
        t_cf = nc.vector.tensor_copy(tv_cf[0:1, 0:1], cf_sb[0:1, 0:1])
        ta_cf = work.tile([128, 1], F32)
        a_cf = nc.scalar.copy(out=ta_cf[:, 0:1], in_=cf_sb[:, 0:1])

        # ---- epilogue ----
        # Compute-engine APs must start at partition 0/32/64/96, so all
        # cross-row arithmetic happens after transposing to the free dim.
        # P4 rows: 0-1 pooled(s0,s1), 2-3 first, 4-5 second, 6-7 ending
        P4 = work.tile([8, H], F32)
        p4op = nc.vector.tensor_scalar_mul(out=P4[:], in0=pooled_ps[:], scalar1=invc_v)
        add_dep_helper(p4op.ins, t_cf.ins, sync=False, reason="cf touch first")

        # LayerNorm stats on pooled rows [0:2]; xn = (pooled - mu) * rstd
        stats = work.tile([2, 2, 6], F32)
        nc.vector.bn_stats(out=stats[:, 0, :], in_=P4[0:2, 0:512])
        nc.vector.bn_stats(out=stats[:, 1, :], in_=P4[0:2, 512:1024])
        mv = work.tile([2, 2], F32)
        nc.vector.bn_aggr(out=mv[:], in_=stats[:])
        eps_sb = work.tile([2, 1], F32)
        nc.vector.memset(eps_sb[:], LN_EPS)
        rstd = work.tile([2, 1], F32)
        nc.scalar.activation(
            out=rstd[:], in_=mv[:, 1:2],
            func=mybir.ActivationFunctionType.Sqrt, bias=eps_sb[:], scale=1.0,
        )
        nc.vector.reciprocal(rstd[:], rstd[:])
        xn = work.tile([2, H], F32)
        nc.vector.tensor_scalar(
            out=xn[:], in0=P4[0:2, :], scalar1=mv[:, 0:1], scalar2=rstd[:],
            op0=mybir.AluOpType.subtract, op1=mybir.AluOpType.mult,
        )

        # XTR[:, 10c + r]: r in 0..8 = P4 row r, r in 8..10 = xn row r-8,
        # for H positions c*128..(c+1)*128 on partitions.
        xtr_ps = pssm.tile([128, 80], F32)
        first_tr = None
        for cc in range(8):
            tr = nc.tensor.transpose(
                out=xtr_ps[:, cc * 10:cc * 10 + 8],
                in_=P4[:, cc * 128:(cc + 1) * 128],
                identity=id8_v,
            )
            if first_tr is None:
                first_tr = tr
                for a in (cb_abs, cf_abs):
                    add_dep_helper(first_tr.ins, a.ins, sync=False, reason="absorbers before epilogue")
            nc.tensor.transpose(
                out=xtr_ps[:, cc * 10 + 8:cc * 10 + 10],
                in_=xn[:, cc * 128:(cc + 1) * 128],
                identity=i2_v,
            )
        XTR = work.tile([128, 8, 10], BF16)
        nc.vector.tensor_copy(XTR[:], xtr_ps[:].rearrange("p (c r) -> p c r", r=10))

        # head inputs on the free dim: esc = relu(second-first), res = relu(-d)
        dT = work.tile([128, 8, 2], BF16)
        nc.vector.tensor_sub(dT[:], XTR[:, :, 4:6], XTR[:, :, 2:4])
        escT = work.tile([128, 8, 2], BF16)
        nc.vector.tensor_scalar_max(out=escT[:], in0=dT[:], scalar1=0.0)
        resT = work.tile([128, 8, 2], BF16)
        nc.vector.tensor_scalar(
            out=resT[:], in0=dT[:], scalar1=-1.0, scalar2=0.0,
            op0=mybir.AluOpType.mult, op1=mybir.AluOpType.max,
        )

        def head_rhs(h, cc):
            if h == 0:
                return escT[:, cc, :]
            if h == 1:
                return resT[:, cc, :]
            if h == 2:
                return XTR[:, cc, 6:8]
            return XTR[:, cc, 8:10]

        # head first layers: h1[:, 2h+j] = w1_h.T @ x_{h,j}
        h1_ps = pssm.tile([128, 8], F32)
        for h in range(4):
            for cc in range(8):
                nc.tensor.matmul(
                    h1_ps[:, 2 * h:2 * h + 2],
                    lhsT=w1_v(h, cc),
                    rhs=head_rhs(h, cc),
                    start=(cc == 0),
                    stop=(cc == 7),
                )
        # exact GELU: z=x+b1, y=erf(z/sqrt(2)), g=z+z*y  (0.5 folded into mh)
        z1 = work.tile([128, 8], F32)
        y1 = work.tile([128, 8], F32)
        for h in range(4):
            sl = slice(2 * h, 2 * h + 2)
            y1op = nc.scalar.activation(
                out=y1[:, sl], in_=h1_ps[:, sl],
                func=mybir.ActivationFunctionType.Erf,
                bias=b1s_v(h), scale=RS2,
            )
            if h == 0:
                add_dep_helper(y1op.ins, a_cf.ins, sync=False, reason="cf act touch first")
            nc.scalar.activation(
                out=z1[:, sl], in_=h1_ps[:, sl],
                func=mybir.ActivationFunctionType.Identity,
                bias=b1_v(h), scale=1.0,
            )
        t1 = work.tile([128, 8], F32)
        nc.vector.tensor_mul(t1[:], z1[:], y1[:])
        g1 = work.tile([128, 8], BF16)
        nc.vector.tensor_add(g1[:], z1[:], t1[:])

        # fc1[:, 2m+j] = fc_w1[:1024,mslice].T @ pooled_j + sum_h mh_h[mslice].T @ g1_{h,j}
        fc1_ps = pssm.tile([128, 4], F32)
        for m in range(2):
            sl = slice(2 * m, 2 * m + 2)
            for cc in range(8):
                nc.tensor.matmul(
                    fc1_ps[:, sl],
                    lhsT=fw1_v(cc, m),
                    rhs=XTR[:, cc, 8:10],
                    start=(cc == 0),
                    stop=False,
                )
            for h in range(4):
                nc.tensor.matmul(
                    fc1_ps[:, sl],
                    lhsT=mh_v(h, m),
                    rhs=g1[:, 2 * h:2 * h + 2],
                    start=False,
                    stop=(h == 3),
                )
        z2 = work.tile([128, 4], F32)
        y2 = work.tile([128, 4], F32)
        for m in range(2):
            sl = slice(2 * m, 2 * m + 2)
            nc.scalar.activation(
                out=y2[:, sl], in_=fc1_ps[:, sl],
                func=mybir.ActivationFunctionType.Erf,
                bias=fb1s_v(m), scale=RS2,
            )
            nc.scalar.activation(
                out=z2[:, sl], in_=fc1_ps[:, sl],
                func=mybir.ActivationFunctionType.Identity,
                bias=fb1_v(m), scale=1.0,
            )
        t2 = work.tile([128, 4], F32)
        nc.vector.tensor_mul(t2[:], z2[:], y2[:])
        g2 = work.tile([128, 4], BF16)
        nc.vector.tensor_add(g2[:], z2[:], t2[:])

        out_ps = pssm.tile([5, 2], F32)
        for m in range(2):
            nc.tensor.matmul(
                out_ps[:],
                lhsT=fw2_v(m),
                rhs=g2[:, 2 * m:2 * m + 2],
                start=(m == 0),
                stop=(m == 1),
            )
        out_sb = work.tile([5, 2], F32)
        oadd = nc.vector.tensor_scalar_add(out=out_sb[:], in0=out_ps[:], scalar1=fb2_v)
        add_dep_helper(oadd.ins, t_cf.ins, sync=False, reason="cf touch first")
        nc.sync.dma_start(out=out_d[:, :], in_=out_sb[:])

    nc.compile()
    return nc


def _pack_k_major(w, k, m):
    """[K, M] -> [128, (K//128)*M] with lhsT chunk c at cols [c*M, (c+1)*M)."""
    return np.ascontiguousarray(
        w.reshape(k // 128, 128, m).transpose(1, 0, 2).reshape(128, (k // 128) * m)
    ).astype(np.float32)


def _host_prep(inputs):
    """Build all per-core in_maps from the full inputs."""
    f32 = np.float32
    bf16 = ml_dtypes.bfloat16
    am = np.asarray(inputs["attention_mask"])
    hid = np.asarray(inputs["hidden"], dtype=f32)

    m_full = am.astype(f32)                      # [B, S]
    L = am.astype(np.int64).sum(1)               # [B]
    pos = np.arange(S)[None, :]
    mid = (L // 2)[:, None]
    Lb = L[:, None]
    st = np.maximum(1, L - 64)[:, None]
    fm = ((pos >= 1) & (pos < mid)).astype(f32)
    sm = ((pos >= mid) & (pos < Lb - 1)).astype(f32)
    em = ((pos >= st) & (pos < Lb - 1)).astype(f32)
    masks = [m_full, fm, sm, em]                 # type order: pooled,first,second,ending
    invs = [
        (1.0 / np.maximum(mk.sum(1, dtype=np.float64), EPS)).astype(f32)
        for mk in masks
    ]

    ln_g = np.asarray(inputs["ln_g"], np.float64)
    ln_b = np.asarray(inputs["ln_b"], np.float64)

    fc_w1 = np.asarray(inputs["fc_w1"], f32)     # [H+4, 256]
    fc_b1 = np.asarray(inputs["fc_b1"], f32)
    fc_w2 = np.asarray(inputs["fc_w2"], f32)     # [256, 5]
    fc_b2 = np.asarray(inputs["fc_b2"], f32)

    # packed const blocks
    cf = np.zeros((128, CF_COLS), f32)
    cf[0:8, CF_ID8:CF_ID8 + 8] = np.eye(8, dtype=f32)
    cf[0:5, CF_FB2] = fc_b2
    cb = np.zeros((128, CB_COLS), bf16)

    fb1_eff = fc_b1.astype(np.float64) + ln_b @ fc_w1[:H].astype(np.float64)
    for h, name in enumerate(HEADS):
        w1 = np.asarray(inputs[f"{name}_w1"], f32).astype(np.float64)  # [H, 128]
        b1 = np.asarray(inputs[f"{name}_b1"], f32).astype(np.float64)  # [128]
        w2 = np.asarray(inputs[f"{name}_w2"], f32)   # [128, 1]
        b2 = np.asarray(inputs[f"{name}_b2"], f32)   # [1]
        if name == "thr":
            # fold the LayerNorm affine into the thr head input weights
            b1 = b1 + ln_b @ w1
            w1 = ln_g[:, None] * w1
        cb[:, CB_W1 + 1024 * h:CB_W1 + 1024 * (h + 1)] = _pack_k_major(
            w1.astype(f32), H, 128
        ).astype(bf16)
        cf[:, CF_B1 + 2 * h] = b1.astype(f32)
        cf[:, CF_B1 + 2 * h + 1] = (b1 * RS2).astype(f32)
        cb[:, CB_MH + 256 * h:CB_MH + 256 * (h + 1)] = np.ascontiguousarray(
            0.5 * w2[:, 0][:, None] * fc_w1[H + h, :][None, :]
        ).astype(bf16)
        fb1_eff = fb1_eff + b2[0] * fc_w1[H + h, :].astype(np.float64)

    fw1_folded = (ln_g[:, None] * fc_w1[:H].astype(np.float64)).astype(f32)
    cb[:, CB_FW1:CB_FW1 + 2048] = _pack_k_major(fw1_folded, H, 256).astype(bf16)
    cb[:, CB_FW2:CB_FW2 + 10] = _pack_k_major(0.5 * fc_w2, 256, 5).astype(bf16)
    fb1_eff = fb1_eff.astype(f32)
    cf[:, CF_FB1 + 0] = fb1_eff[0:128]
    cf[:, CF_FB1 + 1] = fb1_eff[128:256]
    cf[:, CF_FB1 + 2] = fb1_eff[0:128] * RS2
    cf[:, CF_FB1 + 3] = fb1_eff[128:256] * RS2

    in_maps = []
    for i in range(NCORES):
        msk = np.zeros((BPC, S // 128, 128, 8), f32)
        cf_i = cf.copy()
        for b in range(BPC):
            gb = BPC * i + b
            for ty in range(4):
                msk[b, :, :, 2 * ty + b] = masks[ty][gb].reshape(S // 128, 128)
                cf_i[2 * ty + b, CF_INVC] = invs[ty][gb]
        wm = np.ascontiguousarray(
            msk.reshape(NK, 128, 8).transpose(1, 0, 2).reshape(128, NK * 8)
        ).astype(bf16)
        in_maps.append(
            dict(
                hid=np.ascontiguousarray(hid[BPC * i:BPC * (i + 1)]).astype(bf16),
                wm=wm,
                cb=cb,
                cf=cf_i,
            )
        )
    return in_maps


def kernel(**inputs):
    if "nc" not in _NC_CACHE:
        _NC_CACHE["nc"] = _build_nc()
    nc = _NC_CACHE["nc"]
    in_maps = _host_prep(inputs)
    res = run_bass_kernel_spmd(nc, in_maps, core_ids=list(range(NCORES)))
    out = np.empty((B, 5), np.float32)
    for i in range(NCORES):
        out[BPC * i:BPC * (i + 1)] = res.results[i]["out"].T
    return out


# revision 30
# speedup vs baseline: 2.1121x; 1.0755x over previous
"""Trainium2 Bass kernel for nn_DirectionalMultiHeadClassifier.

Data-parallel over 8 NeuronCores: each core handles 2 of the 16 samples.

Math per sample (mirrors the reference):
  - 4 masked means over S of hidden [S,H]: full attention_mask, and three
    position-range masks derived from L = mask.sum() (first/second/ending).
    Computed on-device as one PSUM-accumulated matmul:
        pooled4[8, H] += W_chunk[128, 8].T @ hidden_chunk[128, H]
    where W is a host-built 0/1 mask matrix (4 mask types x 2 samples) and
    the 1/count normalization is applied afterwards.
  - LayerNorm on the full-mask pooled vector; ln_g/ln_b are folded on the
    host into every consumer of the normalized vector (thr head w1/b1 and
    the fc pooled-part weights/bias), so the device only normalizes.
  - 4 small MLP heads (H->128 -> exact GELU -> 128->1). The scalar head
    outputs only feed the final classifier's last 4 input features, so the
    128->1 layer is folded into the classifier on the host:
        fc1 += gelu_h @ (0.5 * w2_h outer fc_w1[1024+h, :])
        fc_b1_eff = fc_b1 + sum_h b2_h * fc_w1[1024+h, :]
  - Final classifier (1028->256 -> exact GELU -> 256->5).
  Exact GELU is computed as 0.5*z*(1+erf(z/sqrt(2))) with the 0.5 folded
  into the following layer's weights.  Every linear bias is applied as a
  K=1 rank-1 matmul (bias_row outer ones) accumulated into PSUM, so the
  GELU needs just one Erf activation per layer.

Compute dtype: hidden/masks/weights stream through the PE in bf16 (masks
are exact 0/1 in bf16); all accumulation is f32 in PSUM.
"""

import ml_dtypes
import numpy as np

import concourse.bass as bass
import concourse.tile as tile
from bass_rust import add_dep_helper
from concourse import bacc, mybir
from concourse.bass_utils import run_bass_kernel_spmd

B, S, H = 16, 2048, 1024
NCORES = 8
BPC = B // NCORES          # samples per core
NK = BPC * (S // 128)      # 128-row contraction chunks per core
TS = 1024                  # S rows per hidden DMA tile (2 MiB bf16)
NT = S // TS               # DMA tiles per sample
RS2 = 0.7071067811865476   # 1/sqrt(2)
LN_EPS = 1e-5
EPS = 1e-9
F32 = mybir.dt.float32
BF16 = mybir.dt.bfloat16
HEADS = ["esc", "res", "end", "thr"]

# packed bf16 const-block column offsets
CB_W1 = 0                  # 4 x [128, 1024]
CB_MH = 4096               # 4 x [128, 256]
CB_FW1 = 5120              # [128, 2048]
CB_FW2 = 7168              # [128, 10]
CB_B1R = 7178              # 4 x [1, 128] bias rows (row 0)
CB_FB1R = 7690             # 2 x [1, 128] fc bias rows (row 0)
CB_FB2R = 7946             # [1, 5] out bias row (row 0)
CB_ONES = 7951             # [1, 2] ones (row 0)
CB_COLS = 7953
# packed f32 const-block column offsets
CF_INVC = 0                # [8, 1]
CF_ID8 = 1                 # [8, 8]
CF_ZERO = 9                # [128, 1] zeros (activation bias)
CF_COLS = 10

_NC_CACHE = {}


def _build_nc():
    """Build the per-core Bass program (identical on all 8 cores)."""
    from contextlib import ExitStack

    nc = bacc.Bacc(
        "TRN2", target_bir_lowering=False, debug=False, num_devices=NCORES
    )
    dp = nc.declare_dram_parameter
    hid_d = dp("hid", [BPC, S, H], BF16, isOutput=False)
    wm_d = dp("wm", [128, NK * 8], BF16, isOutput=False)
    cb_d = dp("cb", [128, CB_COLS], BF16, isOutput=False)
    cf_d = dp("cf", [128, CF_COLS], F32, isOutput=False)
    out_d = dp("out", [5, BPC], F32, isOutput=True)

    with tile.TileContext(nc) as tc, ExitStack() as ctx:
        const = ctx.enter_context(tc.tile_pool(name="const", bufs=1))
        hidp = ctx.enter_context(tc.tile_pool(name="hidp", bufs=BPC * NT))
        work = ctx.enter_context(tc.tile_pool(name="work", bufs=1))
        psmain = ctx.enter_context(tc.tile_pool(name="psmain", bufs=1, space="PSUM"))
        pssm = ctx.enter_context(tc.tile_pool(name="pssm", bufs=1, space="PSUM"))

        # ACT table warm-up: touch the activation functions used later so the
        # ~1.3us/table loads overlap the initial DMAs instead of serializing
        # into the epilogue.
        ws_in = work.tile([1, 1], F32)
        ws_b = work.tile([1, 1], F32)
        ws_out = work.tile([1, 1], F32)
        nc.vector.memset(ws_in[:], 0.0)
        nc.vector.memset(ws_b[:], 0.0)
        for fn in (
            mybir.ActivationFunctionType.Erf,
            mybir.ActivationFunctionType.Sqrt,
        ):
            nc.scalar.activation(out=ws_out[:], in_=ws_in[:], func=fn, bias=ws_b[:])

        # All large DMAs go on the single sync HWDGE ring, explicitly chained
        # so they transfer strictly in this order: wm, tile1..3, consts,
        # tile4.  Sequential transfers hand each tile over ASAP (concurrent
        # round-robin would delay the FIRST tile by 4x) and the params arrive
        # right before the epilogue needs them.
        wm_sb = const.tile([128, NK * 8], BF16, name="c_wm", tag="c_wm")
        cb_sb = const.tile([128, CB_COLS], BF16, name="c_cb", tag="c_cb")
        cf_sb = const.tile([128, CF_COLS], F32, name="c_cf", tag="c_cf")
        dma_chain = [nc.sync.dma_start(out=wm_sb[:], in_=wm_d[:])]

        # const views
        invc_v = cf_sb[0:8, CF_INVC:CF_INVC + 1]
        id8_v = cf_sb[0:8, CF_ID8:CF_ID8 + 8]
        i2_v = cf_sb[0:2, CF_ID8:CF_ID8 + 2]
        zero_v = cf_sb[:, CF_ZERO:CF_ZERO + 1]
        w1_v = lambda h, c: cb_sb[:, CB_W1 + 1024 * h + 128 * c:CB_W1 + 1024 * h + 128 * (c + 1)]
        mh_v = lambda h, m: cb_sb[:, CB_MH + 256 * h + 128 * m:CB_MH + 256 * h + 128 * (m + 1)]
        fw1_v = lambda c, m: cb_sb[:, CB_FW1 + 256 * c + 128 * m:CB_FW1 + 256 * c + 128 * (m + 1)]
        fw2_v = lambda m: cb_sb[:, CB_FW2 + 5 * m:CB_FW2 + 5 * (m + 1)]
        b1r_v = lambda h: cb_sb[0:1, CB_B1R + 128 * h:CB_B1R + 128 * (h + 1)]
        fb1r_v = lambda m: cb_sb[0:1, CB_FB1R + 128 * m:CB_FB1R + 128 * (m + 1)]
        fb2r_v = cb_sb[0:1, CB_FB2R:CB_FB2R + 5]
        ones_v = cb_sb[0:1, CB_ONES:CB_ONES + 2]

        # Wait-absorbers: every engine instruction carries at most ONE
        # semaphore wait in this walrus build, so consume each const DMA's
        # completion once per reading engine; real consumers then only wait
        # on their data inputs.
        scr_ps = pssm.tile([8, 8], F32)

        def absorb(csb):
            return nc.tensor.matmul(
                scr_ps[:, :], lhsT=csb[:, 0:8], rhs=csb[:, 0:8],
                start=True, stop=True,
            )

        wm_abs = absorb(wm_sb)

        # ---- main loop: pooled4[j, h] = sum_s wm[s, j] * hidden[s, h] ----
        pooled_ps = psmain.tile([8, H], F32)
        first_mm = None
        last_mm = None
        tiles = [(b, t) for b in range(BPC) for t in range(NT)]
        for k, (b, t) in enumerate(tiles):
            if k == len(tiles) - 1:
                # params transfer right before the last hidden tile: they are
                # only needed by the epilogue
                dma_chain.append(nc.sync.dma_start(out=cf_sb[:], in_=cf_d[:]))
                dma_chain.append(nc.sync.dma_start(out=cb_sb[:], in_=cb_d[:]))
            ht = hidp.tile([128, TS // 128, H], BF16)
            dma_chain.append(
                nc.sync.dma_start(
                    out=ht[:],
                    in_=hid_d[b, t * TS:(t + 1) * TS, :].rearrange(
                        "(c p) h -> p c h", p=128
                    ),
                )
            )
            for c in range(TS // 128):
                n = b * (S // 128) + t * (TS // 128) + c
                lw = wm_sb[:, n * 8:(n + 1) * 8]
                for j in range(2):
                    mm = nc.tensor.matmul(
                        pooled_ps[:, j * 512:(j + 1) * 512],
                        lhsT=lw,
                        rhs=ht[:, c, j * 512:(j + 1) * 512],
                        start=(n == 0),
                        stop=(n == NK - 1),
                    )
                    if first_mm is None:
                        first_mm = mm
                    last_mm = mm

        for k in range(1, len(dma_chain)):
            add_dep_helper(
                dma_chain[k].ins, dma_chain[k - 1].ins, sync=False,
                reason="serialize sync-ring DMAs",
            )
        add_dep_helper(first_mm.ins, wm_abs.ins, sync=False, reason="absorb wm dma wait")

        # absorbers/touches for epilogue consts; PE ones anchored after the
        # main loop (PE is idle during the DVE epilogue prologue).
        cb_abs = absorb(cb_sb)
        cf_abs = absorb(cf_sb)
        for a in (cb_abs, cf_abs):
            add_dep_helper(a.ins, last_mm.ins, sync=False, reason="absorber after main loop")
        tv_cf = work.tile([1, 1], F32)
        t_cf = nc.vector.tensor_copy(tv_cf[0:1, 0:1], cf_sb[0:1, 0:1])
        ta_cf = work.tile([128, 1], F32)
        a_cf = nc.scalar.copy(out=ta_cf[:, 0:1], in_=cf_sb[:, 0:1])

        # ---- epilogue ----
        # Compute-engine APs must start at partition 0/32/64/96, so all
        # cross-row arithmetic happens after transposing to the free dim.
        # P4 rows: 0-1 pooled(s0,s1), 2-3 first, 4-5 second, 6-7 ending
        P4 = work.tile([8, H], F32)
        p4op = nc.vector.tensor_scalar_mul(out=P4[:], in0=pooled_ps[:], scalar1=invc_v)
        add_dep_helper(p4op.ins, t_cf.ins, sync=False, reason="cf touch first")

        # LayerNorm stats on pooled rows [0:2]; xn = (pooled - mu) * rstd
        stats = work.tile([2, 2, 6], F32)
        nc.vector.bn_stats(out=stats[:, 0, :], in_=P4[0:2, 0:512])
        nc.vector.bn_stats(out=stats[:, 1, :], in_=P4[0:2, 512:1024])
        mv = work.tile([2, 2], F32)
        nc.vector.bn_aggr(out=mv[:], in_=stats[:])
        eps_sb = work.tile([2, 1], F32)
        nc.vector.memset(eps_sb[:], LN_EPS)
        rstd = work.tile([2, 1], F32)
        nc.scalar.activation(
            out=rstd[:], in_=mv[:, 1:2],
            func=mybir.ActivationFunctionType.Sqrt, bias=eps_sb[:], scale=1.0,
        )
        nc.vector.reciprocal(rstd[:], rstd[:])
        xn = work.tile([2, H], F32)
        nc.vector.tensor_scalar(
            out=xn[:], in0=P4[0:2, :], scalar1=mv[:, 0:1], scalar2=rstd[:],
            op0=mybir.AluOpType.subtract, op1=mybir.AluOpType.mult,
        )

        # XTR[:, 10c + r]: r in 0..8 = P4 row r, r in 8..10 = xn row r-8,
        # for H positions c*128..(c+1)*128 on partitions.
        xtr_ps = pssm.tile([128, 80], F32)
        first_tr = None
        for cc in range(8):
            tr = nc.tensor.transpose(
                out=xtr_ps[:, cc * 10:cc * 10 + 8],
                in_=P4[:, cc * 128:(cc + 1) * 128],
                identity=id8_v,
            )
            if first_tr is None:
                first_tr = tr
                for a in (cb_abs, cf_abs):
                    add_dep_helper(first_tr.ins, a.ins, sync=False, reason="absorbers before epilogue")
            nc.tensor.transpose(
                out=xtr_ps[:, cc * 10 + 8:cc * 10 + 10],
                in_=xn[:, cc * 128:(cc + 1) * 128],
                identity=i2_v,
            )
        # per-chunk copies so head matmuls can start before the whole copy
        XTR = work.tile([128, 8, 10], BF16)
        for cc in range(8):
            nc.vector.tensor_copy(XTR[:, cc, :], xtr_ps[:, cc * 10:(cc + 1) * 10])

        # head inputs on the free dim: esc = relu(second-first), res = relu(-d)
        dT = work.tile([128, 8, 2], BF16)
        nc.vector.tensor_sub(dT[:], XTR[:, :, 4:6], XTR[:, :, 2:4])
        escT = work.tile([128, 8, 2], BF16)
        nc.vector.tensor_scalar_max(out=escT[:], in0=dT[:], scalar1=0.0)
        resT = work.tile([128, 8, 2], BF16)
        nc.vector.tensor_scalar(
            out=resT[:], in0=dT[:], scalar1=-1.0, scalar2=0.0,
            op0=mybir.AluOpType.mult, op1=mybir.AluOpType.max,
        )

        def head_rhs(h, cc):
            if h == 0:
                return escT[:, cc, :]
            if h == 1:
                return resT[:, cc, :]
            if h == 2:
                return XTR[:, cc, 6:8]
            return XTR[:, cc, 8:10]

        # head first layers: h1[:, 2h+j] = b1_h + w1_h.T @ x_{h,j}
        h1_ps = pssm.tile([128, 8], F32)
        for h in range(4):
            nc.tensor.matmul(
                h1_ps[:, 2 * h:2 * h + 2], lhsT=b1r_v(h), rhs=ones_v,
                start=True, stop=False,
            )
            for cc in range(8):
                nc.tensor.matmul(
                    h1_ps[:, 2 * h:2 * h + 2],
                    lhsT=w1_v(h, cc),
                    rhs=head_rhs(h, cc),
                    start=False,
                    stop=(cc == 7),
                )
        # exact GELU: y=erf(z/sqrt(2)), g=z+z*y  (0.5 folded into mh weights)
        y1 = work.tile([128, 8], F32)
        y1op = nc.scalar.activation(
            out=y1[:], in_=h1_ps[:],
            func=mybir.ActivationFunctionType.Erf, bias=zero_v, scale=RS2,
        )
        add_dep_helper(y1op.ins, a_cf.ins, sync=False, reason="cf act touch first")
        t1 = work.tile([128, 8], F32)
        nc.vector.tensor_mul(t1[:], h1_ps[:], y1[:])
        g1 = work.tile([128, 8], BF16)
        nc.vector.tensor_add(g1[:], h1_ps[:], t1[:])

        # fc1[:, 2m+j] = fb1 + fc_w1.T @ pooled_j + sum_h mh_h.T @ g1_{h,j}
        fc1_ps = pssm.tile([128, 4], F32)
        for m in range(2):
            sl = slice(2 * m, 2 * m + 2)
            nc.tensor.matmul(
                fc1_ps[:, sl], lhsT=fb1r_v(m), rhs=ones_v,
                start=True, stop=False,
            )
            for cc in range(8):
                nc.tensor.matmul(
                    fc1_ps[:, sl],
                    lhsT=fw1_v(cc, m),
                    rhs=XTR[:, cc, 8:10],
                    start=False,
                    stop=False,
                )
            for h in range(4):
                nc.tensor.matmul(
                    fc1_ps[:, sl],
                    lhsT=mh_v(h, m),
                    rhs=g1[:, 2 * h:2 * h + 2],
                    start=False,
                    stop=(h == 3),
                )
        y2 = work.tile([128, 4], F32)
        nc.scalar.activation(
            out=y2[:], in_=fc1_ps[:],
            func=mybir.ActivationFunctionType.Erf, bias=zero_v, scale=RS2,
        )
        t2 = work.tile([128, 4], F32)
        nc.vector.tensor_mul(t2[:], fc1_ps[:], y2[:])
        g2 = work.tile([128, 4], BF16)
        nc.vector.tensor_add(g2[:], fc1_ps[:], t2[:])

        out_ps = pssm.tile([5, 2], F32)
        nc.tensor.matmul(out_ps[:], lhsT=fb2r_v, rhs=ones_v, start=True, stop=False)
        for m in range(2):
            nc.tensor.matmul(
                out_ps[:],
                lhsT=fw2_v(m),
                rhs=g2[:, 2 * m:2 * m + 2],
                start=False,
                stop=(m == 1),
            )
        out_sb = work.tile([5, 2], F32)
        nc.vector.tensor_copy(out_sb[:], out_ps[:])
        nc.sync.dma_start(out=out_d[:, :], in_=out_sb[:])

    nc.compile()
    return nc


def _pack_k_major(w, k, m):
    """[K, M] -> [128, (K//128)*M] with lhsT chunk c at cols [c*M, (c+1)*M)."""
    return np.ascontiguousarray(
        w.reshape(k // 128, 128, m).transpose(1, 0, 2).reshape(128, (k // 128) * m)
    ).astype(np.float32)


def _host_prep(inputs):
    """Build all per-core in_maps from the full inputs."""
    f32 = np.float32
    bf16 = ml_dtypes.bfloat16
    am = np.asarray(inputs["attention_mask"])
    hid = np.asarray(inputs["hidden"], dtype=f32)

    m_full = am.astype(f32)                      # [B, S]
    L = am.astype(np.int64).sum(1)               # [B]
    pos = np.arange(S)[None, :]
    mid = (L // 2)[:, None]
    Lb = L[:, None]
    st = np.maximum(1, L - 64)[:, None]
    fm = ((pos >= 1) & (pos < mid)).astype(f32)
    sm = ((pos >= mid) & (pos < Lb - 1)).astype(f32)
    em = ((pos >= st) & (pos < Lb - 1)).astype(f32)
    masks = [m_full, fm, sm, em]                 # type order: pooled,first,second,ending
    invs = [
        (1.0 / np.maximum(mk.sum(1, dtype=np.float64), EPS)).astype(f32)
        for mk in masks
    ]

    ln_g = np.asarray(inputs["ln_g"], np.float64)
    ln_b = np.asarray(inputs["ln_b"], np.float64)

    fc_w1 = np.asarray(inputs["fc_w1"], f32)     # [H+4, 256]
    fc_b1 = np.asarray(inputs["fc_b1"], f32)
    fc_w2 = np.asarray(inputs["fc_w2"], f32)     # [256, 5]
    fc_b2 = np.asarray(inputs["fc_b2"], f32)

    # packed const blocks
    cf = np.zeros((128, CF_COLS), f32)
    cf[0:8, CF_ID8:CF_ID8 + 8] = np.eye(8, dtype=f32)
    cb = np.zeros((128, CB_COLS), bf16)
    cb[0, CB_FB2R:CB_FB2R + 5] = fc_b2.astype(bf16)
    cb[0, CB_ONES:CB_ONES + 2] = np.ones(2, bf16)

    fb1_eff = fc_b1.astype(np.float64) + ln_b @ fc_w1[:H].astype(np.float64)
    for h, name in enumerate(HEADS):
        w1 = np.asarray(inputs[f"{name}_w1"], f32).astype(np.float64)  # [H, 128]
        b1 = np.asarray(inputs[f"{name}_b1"], f32).astype(np.float64)  # [128]
        w2 = np.asarray(inputs[f"{name}_w2"], f32)   # [128, 1]
        b2 = np.asarray(inputs[f"{name}_b2"], f32)   # [1]
        if name == "thr":
            # fold the LayerNorm affine into the thr head input weights
            b1 = b1 + ln_b @ w1
            w1 = ln_g[:, None] * w1
        cb[:, CB_W1 + 1024 * h:CB_W1 + 1024 * (h + 1)] = _pack_k_major(
            w1.astype(f32), H, 128
        ).astype(bf16)
        cb[0, CB_B1R + 128 * h:CB_B1R + 128 * (h + 1)] = b1.astype(bf16)
        cb[:, CB_MH + 256 * h:CB_MH + 256 * (h + 1)] = np.ascontiguousarray(
            0.5 * w2[:, 0][:, None] * fc_w1[H + h, :][None, :]
        ).astype(bf16)
        fb1_eff = fb1_eff + b2[0] * fc_w1[H + h, :].astype(np.float64)

    fw1_folded = (ln_g[:, None] * fc_w1[:H].astype(np.float64)).astype(f32)
    cb[:, CB_FW1:CB_FW1 + 2048] = _pack_k_major(fw1_folded, H, 256).astype(bf16)
    cb[:, CB_FW2:CB_FW2 + 10] = _pack_k_major(0.5 * fc_w2, 256, 5).astype(bf16)
    fb1_eff = fb1_eff.astype(f32)
    cb[0, CB_FB1R:CB_FB1R + 128] = fb1_eff[0:128].astype(bf16)
    cb[0, CB_FB1R + 128:CB_FB1R + 256] = fb1_eff[128:256].astype(bf16)

    in_maps = []
    for i in range(NCORES):
        msk = np.zeros((BPC, S // 128, 128, 8), f32)
        cf_i = cf.copy()
        for b in range(BPC):
            gb = BPC * i + b
            for ty in range(4):
                msk[b, :, :, 2 * ty + b] = masks[ty][gb].reshape(S // 128, 128)
                cf_i[2 * ty + b, CF_INVC] = invs[ty][gb]
        wm = np.ascontiguousarray(
            msk.reshape(NK, 128, 8).transpose(1, 0, 2).reshape(128, NK * 8)
        ).astype(bf16)
        in_maps.append(
            dict(
                hid=np.ascontiguousarray(hid[BPC * i:BPC * (i + 1)]).astype(bf16),
                wm=wm,
                cb=cb,
                cf=cf_i,
            )
        )
    return in_maps


def kernel(**inputs):
    if "nc" not in _NC_CACHE:
        _NC_CACHE["nc"] = _build_nc()
    nc = _NC_CACHE["nc"]
    in_maps = _host_prep(inputs)
    res = run_bass_kernel_spmd(nc, in_maps, core_ids=list(range(NCORES)))
    out = np.empty((B, 5), np.float32)
    for i in range(NCORES):
        out[BPC * i:BPC * (i + 1)] = res.results[i]["out"].T
    return out


# revision 33
# speedup vs baseline: 2.2538x; 1.0671x over previous
"""Trainium2 Bass kernel for nn_DirectionalMultiHeadClassifier.

Data-parallel over 8 NeuronCores: each core handles 2 of the 16 samples.

Math per sample (mirrors the reference):
  - 4 masked means over S of hidden [S,H]: full attention_mask, and three
    position-range masks derived from L = mask.sum() (first/second/ending).
    Computed on-device as one PSUM-accumulated matmul:
        pooled4[8, H] += W_chunk[128, 8].T @ hidden_chunk[128, H]
    where W is a host-built 0/1 mask matrix (4 mask types x 2 samples) and
    the 1/count normalization is applied afterwards.
  - LayerNorm on the full-mask pooled vector; ln_g/ln_b are folded on the
    host into every consumer of the normalized vector (thr head w1/b1 and
    the fc pooled-part weights/bias), so the device only normalizes.
  - 4 small MLP heads (H->128 -> exact GELU -> 128->1). The scalar head
    outputs only feed the final classifier's last 4 input features, so the
    128->1 layer is folded into the classifier on the host:
        fc1 += gelu_h @ (0.5 * w2_h outer fc_w1[1024+h, :])
        fc_b1_eff = fc_b1 + sum_h b2_h * fc_w1[1024+h, :]
  - Final classifier (1028->256 -> exact GELU -> 256->5).
  Exact GELU is computed as 0.5*z*(1+erf(z/sqrt(2))) with the 0.5 folded
  into the following layer's weights.  Every linear bias is applied as a
  K=1 rank-1 matmul (bias_row outer ones) accumulated into PSUM, so the
  GELU needs just one Erf activation per layer.

Compute dtype: hidden/masks/weights stream through the PE in bf16 (masks
are exact 0/1 in bf16); all accumulation is f32 in PSUM.
"""

import ml_dtypes
import numpy as np

import concourse.bass as bass
import concourse.tile as tile
from bass_rust import add_dep_helper
from concourse import bacc, mybir
from concourse.bass_utils import run_bass_kernel_spmd

B, S, H = 16, 2048, 1024
NCORES = 8
BPC = B // NCORES          # samples per core
NK = BPC * (S // 128)      # 128-row contraction chunks per core
TS = 512                   # S rows per hidden DMA tile (1 MiB bf16)
NT = S // TS               # DMA tiles per sample
RS2 = 0.7071067811865476   # 1/sqrt(2)
LN_EPS = 1e-5
EPS = 1e-9
F32 = mybir.dt.float32
BF16 = mybir.dt.bfloat16
HEADS = ["esc", "res", "end", "thr"]

# packed bf16 const-block column offsets
CB_W1 = 0                  # 4 x [128, 1024]
CB_MH = 4096               # 4 x [128, 256]
CB_FW1 = 5120              # [128, 2048]
CB_FW2 = 7168              # [128, 10]
CB_B1R = 7178              # 4 x [1, 128] bias rows (row 0)
CB_FB1R = 7690             # 2 x [1, 128] fc bias rows (row 0)
CB_FB2R = 7946             # [1, 5] out bias row (row 0)
CB_ONES = 7951             # [1, 2] ones (row 0)
CB_COLS = 7953
# packed f32 const-block column offsets
CF_INVC = 0                # [8, 1]
CF_ID8 = 1                 # [8, 8]
CF_ZERO = 9                # [128, 1] zeros (activation bias)
CF_COLS = 10

_NC_CACHE = {}


def _build_nc():
    """Build the per-core Bass program (identical on all 8 cores)."""
    from contextlib import ExitStack

    nc = bacc.Bacc(
        "TRN2", target_bir_lowering=False, debug=False, num_devices=NCORES
    )
    dp = nc.declare_dram_parameter
    hid_d = dp("hid", [BPC, S, H], BF16, isOutput=False)
    wm_d = dp("wm", [128, NK * 8], BF16, isOutput=False)
    cb_d = dp("cb", [128, CB_COLS], BF16, isOutput=False)
    cf_d = dp("cf", [128, CF_COLS], F32, isOutput=False)
    out_d = dp("out", [5, BPC], F32, isOutput=True)

    with tile.TileContext(nc) as tc, ExitStack() as ctx:
        const = ctx.enter_context(tc.tile_pool(name="const", bufs=1))
        hidp = ctx.enter_context(tc.tile_pool(name="hidp", bufs=BPC * NT))
        work = ctx.enter_context(tc.tile_pool(name="work", bufs=1))
        psmain = ctx.enter_context(tc.tile_pool(name="psmain", bufs=1, space="PSUM"))
        pssm = ctx.enter_context(tc.tile_pool(name="pssm", bufs=1, space="PSUM"))

        # ACT table warm-up: touch the activation functions used later so the
        # ~1.3us/table loads overlap the initial DMAs instead of serializing
        # into the epilogue.
        ws_in = work.tile([1, 1], F32)
        ws_b = work.tile([1, 1], F32)
        ws_out = work.tile([1, 1], F32)
        nc.vector.memset(ws_in[:], 0.0)
        nc.vector.memset(ws_b[:], 0.0)
        for fn in (
            mybir.ActivationFunctionType.Erf,
            mybir.ActivationFunctionType.Sqrt,
        ):
            nc.scalar.activation(out=ws_out[:], in_=ws_in[:], func=fn, bias=ws_b[:])

        # All large DMAs go on the single sync HWDGE ring, explicitly chained
        # so they transfer strictly in this order: wm, tile1..3, consts,
        # tile4.  Sequential transfers hand each tile over ASAP (concurrent
        # round-robin would delay the FIRST tile by 4x) and the params arrive
        # right before the epilogue needs them.
        wm_sb = const.tile([128, NK * 8], BF16, name="c_wm", tag="c_wm")
        cb_sb = const.tile([128, CB_COLS], BF16, name="c_cb", tag="c_cb")
        cf_sb = const.tile([128, CF_COLS], F32, name="c_cf", tag="c_cf")
        dma_chain = [nc.sync.dma_start(out=wm_sb[:], in_=wm_d[:])]

        # const views
        invc_v = cf_sb[0:8, CF_INVC:CF_INVC + 1]
        id8_v = cf_sb[0:8, CF_ID8:CF_ID8 + 8]
        i2_v = cf_sb[0:2, CF_ID8:CF_ID8 + 2]
        zero_v = cf_sb[:, CF_ZERO:CF_ZERO + 1]
        w1_v = lambda h, c: cb_sb[:, CB_W1 + 1024 * h + 128 * c:CB_W1 + 1024 * h + 128 * (c + 1)]
        mh_v = lambda h, m: cb_sb[:, CB_MH + 256 * h + 128 * m:CB_MH + 256 * h + 128 * (m + 1)]
        fw1_v = lambda c, m: cb_sb[:, CB_FW1 + 256 * c + 128 * m:CB_FW1 + 256 * c + 128 * (m + 1)]
        fw2_v = lambda m: cb_sb[:, CB_FW2 + 5 * m:CB_FW2 + 5 * (m + 1)]
        b1r_v = lambda h: cb_sb[0:1, CB_B1R + 128 * h:CB_B1R + 128 * (h + 1)]
        fb1r_v = lambda m: cb_sb[0:1, CB_FB1R + 128 * m:CB_FB1R + 128 * (m + 1)]
        fb2r_v = cb_sb[0:1, CB_FB2R:CB_FB2R + 5]
        ones_v = cb_sb[0:1, CB_ONES:CB_ONES + 2]

        # Wait-absorbers: every engine instruction carries at most ONE
        # semaphore wait in this walrus build, so consume each const DMA's
        # completion once per reading engine; real consumers then only wait
        # on their data inputs.
        scr_ps = pssm.tile([8, 8], F32)

        def absorb(csb):
            return nc.tensor.matmul(
                scr_ps[:, :], lhsT=csb[:, 0:8], rhs=csb[:, 0:8],
                start=True, stop=True,
            )

        wm_abs = absorb(wm_sb)

        # ---- main loop: pooled4[j, h] = sum_s wm[s, j] * hidden[s, h] ----
        pooled_ps = psmain.tile([8, H], F32)
        first_mm = None
        last_mm = None
        tiles = [(b, t) for b in range(BPC) for t in range(NT)]
        for k, (b, t) in enumerate(tiles):
            if k == len(tiles) - 1:
                # params transfer right before the last hidden tile: they are
                # only needed by the epilogue
                dma_chain.append(nc.sync.dma_start(out=cf_sb[:], in_=cf_d[:]))
                dma_chain.append(nc.sync.dma_start(out=cb_sb[:], in_=cb_d[:]))
            ht = hidp.tile([128, TS // 128, H], BF16)
            dma_chain.append(
                nc.sync.dma_start(
                    out=ht[:],
                    in_=hid_d[b, t * TS:(t + 1) * TS, :].rearrange(
                        "(c p) h -> p c h", p=128
                    ),
                )
            )
            for c in range(TS // 128):
                n = b * (S // 128) + t * (TS // 128) + c
                lw = wm_sb[:, n * 8:(n + 1) * 8]
                for j in range(2):
                    mm = nc.tensor.matmul(
                        pooled_ps[:, j * 512:(j + 1) * 512],
                        lhsT=lw,
                        rhs=ht[:, c, j * 512:(j + 1) * 512],
                        start=(n == 0),
                        stop=(n == NK - 1),
                    )
                    if first_mm is None:
                        first_mm = mm
                    last_mm = mm

        for k in range(1, len(dma_chain)):
            add_dep_helper(
                dma_chain[k].ins, dma_chain[k - 1].ins, sync=False,
                reason="serialize sync-ring DMAs",
            )
        add_dep_helper(first_mm.ins, wm_abs.ins, sync=False, reason="absorb wm dma wait")

        # absorbers/touches for epilogue consts; PE ones anchored after the
        # main loop (PE is idle during the DVE epilogue prologue).
        cb_abs = absorb(cb_sb)
        cf_abs = absorb(cf_sb)
        for a in (cb_abs, cf_abs):
            add_dep_helper(a.ins, last_mm.ins, sync=False, reason="absorber after main loop")
        tv_cf = work.tile([1, 1], F32)
        t_cf = nc.vector.tensor_copy(tv_cf[0:1, 0:1], cf_sb[0:1, 0:1])
        ta_cf = work.tile([128, 1], F32)
        a_cf = nc.scalar.copy(out=ta_cf[:, 0:1], in_=cf_sb[:, 0:1])

        # ---- epilogue ----
        # Compute-engine APs must start at partition 0/32/64/96, so all
        # cross-row arithmetic happens after transposing to the free dim.
        # P4 rows: 0-1 pooled(s0,s1), 2-3 first, 4-5 second, 6-7 ending
        # The 1/count scaling runs on ACT (Copy with per-partition scale)
        # while DVE computes the LayerNorm stats straight from raw PSUM:
        # mu' = mu_raw*inv, rstd' = 1/sqrt(var_raw*inv^2 + eps), and
        # xn = (raw - mu_raw) * (inv * rstd').
        P4 = work.tile([8, H], F32)
        p4op = nc.scalar.activation(
            out=P4[:], in_=pooled_ps[:],
            func=mybir.ActivationFunctionType.Copy, bias=0.0, scale=invc_v,
        )
        add_dep_helper(p4op.ins, a_cf.ins, sync=False, reason="cf act touch first")

        stats = work.tile([2, 2, 6], F32)
        nc.vector.bn_stats(out=stats[:, 0, :], in_=pooled_ps[0:2, 0:512])
        nc.vector.bn_stats(out=stats[:, 1, :], in_=pooled_ps[0:2, 512:1024])
        mv = work.tile([2, 2], F32)
        bnop = nc.vector.bn_aggr(out=mv[:], in_=stats[:])
        add_dep_helper(bnop.ins, t_cf.ins, sync=False, reason="cf touch first")
        iv2 = work.tile([2, 1], F32)
        nc.vector.tensor_mul(iv2[:], invc_v[0:2, :], invc_v[0:2, :])
        vsc = work.tile([2, 1], F32)
        nc.vector.tensor_mul(vsc[:], mv[:, 1:2], iv2[:])
        eps_sb = work.tile([2, 1], F32)
        nc.vector.memset(eps_sb[:], LN_EPS)
        rstd = work.tile([2, 1], F32)
        sqop = nc.scalar.activation(
            out=rstd[:], in_=vsc[:],
            func=mybir.ActivationFunctionType.Sqrt, bias=eps_sb[:], scale=1.0,
        )
        # re-warm the Erf table right after the (sole) Sqrt use so the later
        # Erf activations don't pay the table load on the critical chain
        erf_rewarm = nc.scalar.activation(
            out=ws_out[:], in_=ws_in[:],
            func=mybir.ActivationFunctionType.Erf, bias=ws_b[:],
        )
        add_dep_helper(erf_rewarm.ins, sqop.ins, sync=False, reason="erf rewarm after sqrt")
        nc.vector.reciprocal(rstd[:], rstd[:])
        s2 = work.tile([2, 1], F32)
        nc.vector.tensor_mul(s2[:], invc_v[0:2, :], rstd[:])
        xn = work.tile([2, H], F32)
        nc.vector.tensor_scalar(
            out=xn[:], in0=pooled_ps[0:2, :], scalar1=mv[:, 0:1], scalar2=s2[:],
            op0=mybir.AluOpType.subtract, op1=mybir.AluOpType.mult,
        )

        # XTR[:, 10c + r]: r in 0..8 = P4 row r, r in 8..10 = xn row r-8,
        # for H positions c*128..(c+1)*128 on partitions.
        xtr_ps = pssm.tile([128, 80], F32)
        first_tr = None
        for cc in range(8):
            tr = nc.tensor.transpose(
                out=xtr_ps[:, cc * 10:cc * 10 + 8],
                in_=P4[:, cc * 128:(cc + 1) * 128],
                identity=id8_v,
            )
            if first_tr is None:
                first_tr = tr
                for a in (cb_abs, cf_abs):
                    add_dep_helper(first_tr.ins, a.ins, sync=False, reason="absorbers before epilogue")
            nc.tensor.transpose(
                out=xtr_ps[:, cc * 10 + 8:cc * 10 + 10],
                in_=xn[:, cc * 128:(cc + 1) * 128],
                identity=i2_v,
            )
        XTR = work.tile([128, 8, 10], BF16)
        nc.vector.tensor_copy(XTR[:], xtr_ps[:].rearrange("p (c r) -> p c r", r=10))

        # head inputs on the free dim: esc = relu(second-first), res = relu(-d)
        dT = work.tile([128, 8, 2], BF16)
        nc.vector.tensor_sub(dT[:], XTR[:, :, 4:6], XTR[:, :, 2:4])
        escT = work.tile([128, 8, 2], BF16)
        nc.vector.tensor_scalar_max(out=escT[:], in0=dT[:], scalar1=0.0)
        resT = work.tile([128, 8, 2], BF16)
        nc.vector.tensor_scalar(
            out=resT[:], in0=dT[:], scalar1=-1.0, scalar2=0.0,
            op0=mybir.AluOpType.mult, op1=mybir.AluOpType.max,
        )

        def head_rhs(h, cc):
            if h == 0:
                return escT[:, cc, :]
            if h == 1:
                return resT[:, cc, :]
            if h == 2:
                return XTR[:, cc, 6:8]
            return XTR[:, cc, 8:10]

        # head first layers: h1[:, 2h+j] = b1_h + w1_h.T @ x_{h,j}
        h1_ps = pssm.tile([128, 8], F32)
        for h in range(4):
            nc.tensor.matmul(
                h1_ps[:, 2 * h:2 * h + 2], lhsT=b1r_v(h), rhs=ones_v,
                start=True, stop=False,
            )
            for cc in range(8):
                nc.tensor.matmul(
                    h1_ps[:, 2 * h:2 * h + 2],
                    lhsT=w1_v(h, cc),
                    rhs=head_rhs(h, cc),
                    start=False,
                    stop=(cc == 7),
                )
        # exact GELU: y=erf(z/sqrt(2)), g=z+z*y  (0.5 folded into mh weights)
        y1 = work.tile([128, 8], F32)
        y1op = nc.scalar.activation(
            out=y1[:], in_=h1_ps[:],
            func=mybir.ActivationFunctionType.Erf, bias=zero_v, scale=RS2,
        )
        add_dep_helper(y1op.ins, erf_rewarm.ins, sync=False, reason="erf rewarmed first")
        t1 = work.tile([128, 8], F32)
        nc.vector.tensor_mul(t1[:], h1_ps[:], y1[:])
        g1 = work.tile([128, 8], BF16)
        nc.vector.tensor_add(g1[:], h1_ps[:], t1[:])

        # fc1[:, 2m+j] = fb1 + fc_w1.T @ pooled_j + sum_h mh_h.T @ g1_{h,j}
        fc1_ps = pssm.tile([128, 4], F32)
        for m in range(2):
            sl = slice(2 * m, 2 * m + 2)
            nc.tensor.matmul(
                fc1_ps[:, sl], lhsT=fb1r_v(m), rhs=ones_v,
                start=True, stop=False,
            )
            for cc in range(8):
                nc.tensor.matmul(
                    fc1_ps[:, sl],
                    lhsT=fw1_v(cc, m),
                    rhs=XTR[:, cc, 8:10],
                    start=False,
                    stop=False,
                )
            for h in range(4):
                nc.tensor.matmul(
                    fc1_ps[:, sl],
                    lhsT=mh_v(h, m),
                    rhs=g1[:, 2 * h:2 * h + 2],
                    start=False,
                    stop=(h == 3),
                )
        y2 = work.tile([128, 4], F32)
        nc.scalar.activation(
            out=y2[:], in_=fc1_ps[:],
            func=mybir.ActivationFunctionType.Erf, bias=zero_v, scale=RS2,
        )
        t2 = work.tile([128, 4], F32)
        nc.vector.tensor_mul(t2[:], fc1_ps[:], y2[:])
        g2 = work.tile([128, 4], BF16)
        nc.vector.tensor_add(g2[:], fc1_ps[:], t2[:])

        out_ps = pssm.tile([5, 2], F32)
        nc.tensor.matmul(out_ps[:], lhsT=fb2r_v, rhs=ones_v, start=True, stop=False)
        for m in range(2):
            nc.tensor.matmul(
                out_ps[:],
                lhsT=fw2_v(m),
                rhs=g2[:, 2 * m:2 * m + 2],
                start=False,
                stop=(m == 1),
            )
        out_sb = work.tile([5, 2], F32)
        nc.vector.tensor_copy(out_sb[:], out_ps[:])
        nc.gpsimd.dma_start(out=out_d[:, :], in_=out_sb[:])

    nc.compile()
    return nc


def _pack_k_major(w, k, m):
    """[K, M] -> [128, (K//128)*M] with lhsT chunk c at cols [c*M, (c+1)*M)."""
    return np.ascontiguousarray(
        w.reshape(k // 128, 128, m).transpose(1, 0, 2).reshape(128, (k // 128) * m)
    ).astype(np.float32)


def _host_prep(inputs):
    """Build all per-core in_maps from the full inputs."""
    f32 = np.float32
    bf16 = ml_dtypes.bfloat16
    am = np.asarray(inputs["attention_mask"])
    hid = np.asarray(inputs["hidden"], dtype=f32)

    m_full = am.astype(f32)                      # [B, S]
    L = am.astype(np.int64).sum(1)               # [B]
    pos = np.arange(S)[None, :]
    mid = (L // 2)[:, None]
    Lb = L[:, None]
    st = np.maximum(1, L - 64)[:, None]
    fm = ((pos >= 1) & (pos < mid)).astype(f32)
    sm = ((pos >= mid) & (pos < Lb - 1)).astype(f32)
    em = ((pos >= st) & (pos < Lb - 1)).astype(f32)
    masks = [m_full, fm, sm, em]                 # type order: pooled,first,second,ending
    invs = [
        (1.0 / np.maximum(mk.sum(1, dtype=np.float64), EPS)).astype(f32)
        for mk in masks
    ]

    ln_g = np.asarray(inputs["ln_g"], np.float64)
    ln_b = np.asarray(inputs["ln_b"], np.float64)

    fc_w1 = np.asarray(inputs["fc_w1"], f32)     # [H+4, 256]
    fc_b1 = np.asarray(inputs["fc_b1"], f32)
    fc_w2 = np.asarray(inputs["fc_w2"], f32)     # [256, 5]
    fc_b2 = np.asarray(inputs["fc_b2"], f32)

    # packed const blocks
    cf = np.zeros((128, CF_COLS), f32)
    cf[0:8, CF_ID8:CF_ID8 + 8] = np.eye(8, dtype=f32)
    cb = np.zeros((128, CB_COLS), bf16)
    cb[0, CB_FB2R:CB_FB2R + 5] = fc_b2.astype(bf16)
    cb[0, CB_ONES:CB_ONES + 2] = np.ones(2, bf16)

    fb1_eff = fc_b1.astype(np.float64) + ln_b @ fc_w1[:H].astype(np.float64)
    for h, name in enumerate(HEADS):
        w1 = np.asarray(inputs[f"{name}_w1"], f32).astype(np.float64)  # [H, 128]
        b1 = np.asarray(inputs[f"{name}_b1"], f32).astype(np.float64)  # [128]
        w2 = np.asarray(inputs[f"{name}_w2"], f32)   # [128, 1]
        b2 = np.asarray(inputs[f"{name}_b2"], f32)   # [1]
        if name == "thr":
            # fold the LayerNorm affine into the thr head input weights
            b1 = b1 + ln_b @ w1
            w1 = ln_g[:, None] * w1
        cb[:, CB_W1 + 1024 * h:CB_W1 + 1024 * (h + 1)] = _pack_k_major(
            w1.astype(f32), H, 128
        ).astype(bf16)
        cb[0, CB_B1R + 128 * h:CB_B1R + 128 * (h + 1)] = b1.astype(bf16)
        cb[:, CB_MH + 256 * h:CB_MH + 256 * (h + 1)] = np.ascontiguousarray(
            0.5 * w2[:, 0][:, None] * fc_w1[H + h, :][None, :]
        ).astype(bf16)
        fb1_eff = fb1_eff + b2[0] * fc_w1[H + h, :].astype(np.float64)

    fw1_folded = (ln_g[:, None] * fc_w1[:H].astype(np.float64)).astype(f32)
    cb[:, CB_FW1:CB_FW1 + 2048] = _pack_k_major(fw1_folded, H, 256).astype(bf16)
    cb[:, CB_FW2:CB_FW2 + 10] = _pack_k_major(0.5 * fc_w2, 256, 5).astype(bf16)
    fb1_eff = fb1_eff.astype(f32)
    cb[0, CB_FB1R:CB_FB1R + 128] = fb1_eff[0:128].astype(bf16)
    cb[0, CB_FB1R + 128:CB_FB1R + 256] = fb1_eff[128:256].astype(bf16)

    in_maps = []
    for i in range(NCORES):
        msk = np.zeros((BPC, S // 128, 128, 8), f32)
        cf_i = cf.copy()
        for b in range(BPC):
            gb = BPC * i + b
            for ty in range(4):
                msk[b, :, :, 2 * ty + b] = masks[ty][gb].reshape(S // 128, 128)
                cf_i[2 * ty + b, CF_INVC] = invs[ty][gb]
        wm = np.ascontiguousarray(
            msk.reshape(NK, 128, 8).transpose(1, 0, 2).reshape(128, NK * 8)
        ).astype(bf16)
        in_maps.append(
            dict(
                hid=np.ascontiguousarray(hid[BPC * i:BPC * (i + 1)]).astype(bf16),
                wm=wm,
                cb=cb,
                cf=cf_i,
            )
        )
    return in_maps


def kernel(**inputs):
    if "nc" not in _NC_CACHE:
        _NC_CACHE["nc"] = _build_nc()
    nc = _NC_CACHE["nc"]
    in_maps = _host_prep(inputs)
    res = run_bass_kernel_spmd(nc, in_maps, core_ids=list(range(NCORES)))
    out = np.empty((B, 5), np.float32)
    for i in range(NCORES):
        out[BPC * i:BPC * (i + 1)] = res.results[i]["out"].T
    return out


# revision 38
# speedup vs baseline: 2.3117x; 1.0257x over previous
"""Trainium2 Bass kernel for nn_DirectionalMultiHeadClassifier.

Data-parallel over 8 NeuronCores: each core handles 2 of the 16 samples.

Math per sample (mirrors the reference):
  - 4 masked means over S of hidden [S,H]: full attention_mask, and three
    position-range masks derived from L = mask.sum() (first/second/ending).
    Computed on-device as one PSUM-accumulated matmul:
        pooled4[8, H] += W_chunk[128, 8].T @ hidden_chunk[128, H]
    where W is a host-built 0/1 mask matrix (4 mask types x 2 samples) and
    the 1/count normalization is applied afterwards.
  - LayerNorm on the full-mask pooled vector; ln_g/ln_b are folded on the
    host into every consumer of the normalized vector (thr head w1/b1 and
    the fc pooled-part weights/bias), so the device only normalizes.
  - 4 small MLP heads (H->128 -> exact GELU -> 128->1). The scalar head
    outputs only feed the final classifier's last 4 input features, so the
    128->1 layer is folded into the classifier on the host:
        fc1 += gelu_h @ (0.5 * w2_h outer fc_w1[1024+h, :])
        fc_b1_eff = fc_b1 + sum_h b2_h * fc_w1[1024+h, :]
  - Final classifier (1028->256 -> exact GELU -> 256->5).
  Exact GELU is computed as 0.5*z*(1+erf(z/sqrt(2))) with the 0.5 folded
  into the following layer's weights.  Every linear bias is applied as a
  K=1 rank-1 matmul (bias_row outer ones) accumulated into PSUM, so the
  GELU needs just one Erf activation per layer.

Compute dtype: hidden/masks/weights stream through the PE in bf16 (masks
are exact 0/1 in bf16); all accumulation is f32 in PSUM.
"""

import ml_dtypes
import numpy as np

import concourse.bass as bass
import concourse.tile as tile
from bass_rust import add_dep_helper
from concourse import bacc, mybir
from concourse.bass_utils import run_bass_kernel_spmd

B, S, H = 16, 2048, 1024
NCORES = 8
BPC = B // NCORES          # samples per core
NK = BPC * (S // 128)      # 128-row contraction chunks per core
TS = 512                   # S rows per hidden DMA tile (1 MiB bf16)
NT = S // TS               # DMA tiles per sample
RS2 = 0.7071067811865476   # 1/sqrt(2)
LN_EPS = 1e-5
EPS = 1e-9
F32 = mybir.dt.float32
BF16 = mybir.dt.bfloat16
HEADS = ["esc", "res", "end", "thr"]

# packed bf16 const-block column offsets
CB_W1 = 0                  # 4 x [128, 1024]
CB_MH = 4096               # 4 x [128, 256]
CB_FW1 = 5120              # [128, 2048]
CB_FW2 = 7168              # [128, 10]
CB_B1R = 7178              # 4 x [1, 128] bias rows (row 0)
CB_FB1R = 7690             # 2 x [1, 128] fc bias rows (row 0)
CB_FB2R = 7946             # [1, 5] out bias row (row 0)
CB_ONES = 7951             # [1, 2] ones (row 0)
CB_COLS = 7953
# packed f32 const-block column offsets
CF_INVC = 0                # [8, 1]
CF_ID8 = 1                 # [8, 8]
CF_ZERO = 9                # [128, 1] zeros (activation bias)
CF_COLS = 10

_NC_CACHE = {}


def _build_nc():
    """Build the per-core Bass program (identical on all 8 cores)."""
    from contextlib import ExitStack

    nc = bacc.Bacc(
        "TRN2", target_bir_lowering=False, debug=False, num_devices=NCORES
    )
    dp = nc.declare_dram_parameter
    hid_d = dp("hid", [BPC, S, H], BF16, isOutput=False)
    wm_d = dp("wm", [128, NK * 8], BF16, isOutput=False)
    cb_d = dp("cb", [128, CB_COLS], BF16, isOutput=False)
    cf_d = dp("cf", [128, CF_COLS], F32, isOutput=False)
    out_d = dp("out", [5, BPC], F32, isOutput=True)

    with tile.TileContext(nc) as tc, ExitStack() as ctx:
        const = ctx.enter_context(tc.tile_pool(name="const", bufs=1))
        hidp = ctx.enter_context(tc.tile_pool(name="hidp", bufs=BPC * NT))
        work = ctx.enter_context(tc.tile_pool(name="work", bufs=1))
        psmain = ctx.enter_context(tc.tile_pool(name="psmain", bufs=1, space="PSUM"))
        pssm = ctx.enter_context(tc.tile_pool(name="pssm", bufs=1, space="PSUM"))

        # ACT table warm-up: touch the activation functions used later so the
        # ~1.3us/table loads overlap the initial DMAs instead of serializing
        # into the epilogue.
        ws_in = work.tile([1, 1], F32)
        ws_b = work.tile([1, 1], F32)
        ws_out = work.tile([1, 1], F32)
        nc.vector.memset(ws_in[:], 0.0)
        nc.vector.memset(ws_b[:], 0.0)
        for fn in (
            mybir.ActivationFunctionType.Erf,
            mybir.ActivationFunctionType.Sqrt,
        ):
            nc.scalar.activation(out=ws_out[:], in_=ws_in[:], func=fn, bias=ws_b[:])

        # All large DMAs go on the single sync HWDGE ring, explicitly chained
        # so they transfer strictly in this order: wm, tile1..3, consts,
        # tile4.  Sequential transfers hand each tile over ASAP (concurrent
        # round-robin would delay the FIRST tile by 4x) and the params arrive
        # right before the epilogue needs them.
        wm_sb = const.tile([128, NK * 8], BF16, name="c_wm", tag="c_wm")
        cb_sb = const.tile([128, CB_COLS], BF16, name="c_cb", tag="c_cb")
        cf_sb = const.tile([128, CF_COLS], F32, name="c_cf", tag="c_cf")
        dma_chain = [nc.sync.dma_start(out=wm_sb[:], in_=wm_d[:])]

        # const views
        invc_v = cf_sb[0:8, CF_INVC:CF_INVC + 1]
        id8_v = cf_sb[0:8, CF_ID8:CF_ID8 + 8]
        i2_v = cf_sb[0:2, CF_ID8:CF_ID8 + 2]
        zero_v = cf_sb[:, CF_ZERO:CF_ZERO + 1]
        w1_v = lambda h, c: cb_sb[:, CB_W1 + 1024 * h + 128 * c:CB_W1 + 1024 * h + 128 * (c + 1)]
        mh_v = lambda h, m: cb_sb[:, CB_MH + 256 * h + 128 * m:CB_MH + 256 * h + 128 * (m + 1)]
        fw1_v = lambda c, m: cb_sb[:, CB_FW1 + 256 * c + 128 * m:CB_FW1 + 256 * c + 128 * (m + 1)]
        fw2_v = lambda m: cb_sb[:, CB_FW2 + 5 * m:CB_FW2 + 5 * (m + 1)]
        b1r_v = lambda h: cb_sb[0:1, CB_B1R + 128 * h:CB_B1R + 128 * (h + 1)]
        fb1r_v = lambda m: cb_sb[0:1, CB_FB1R + 128 * m:CB_FB1R + 128 * (m + 1)]
        fb2r_v = cb_sb[0:1, CB_FB2R:CB_FB2R + 5]
        ones_v = cb_sb[0:1, CB_ONES:CB_ONES + 2]

        # Wait-absorbers: every engine instruction carries at most ONE
        # semaphore wait in this walrus build, so consume each const DMA's
        # completion once per reading engine; real consumers then only wait
        # on their data inputs.
        scr_ps = pssm.tile([8, 8], F32)

        def absorb(csb):
            return nc.tensor.matmul(
                scr_ps[:, :], lhsT=csb[:, 0:8], rhs=csb[:, 0:8],
                start=True, stop=True,
            )

        # PE warm-up: the HAM clock gate defaults to 1.2 GHz and needs ~3.4us
        # of sustained activity to unthrottle.  Run junk matmuls during the
        # initial DMA wait so the real loop starts (and stays) at 2.4 GHz.
        warm_in = work.tile([128, 256], BF16)
        nc.vector.memset(warm_in[:], 0.0)
        warm_ps = pssm.tile([8, 512], F32)
        warm_last = None
        for _ in range(72):
            warm_last = nc.tensor.matmul(
                warm_ps[:, 0:256], lhsT=warm_in[:, 0:8], rhs=warm_in[:, 0:256],
                start=True, stop=True,
            )

        wm_abs = absorb(wm_sb)
        add_dep_helper(wm_abs.ins, warm_last.ins, sync=False, reason="warmup before wm absorber")

        # ---- main loop: pooled4[j, h] = sum_s wm[s, j] * hidden[s, h] ----
        pooled_ps = psmain.tile([8, H], F32)
        first_mm = None
        last_mm = None
        tiles = [(b, t) for b in range(BPC) for t in range(NT)]
        for k, (b, t) in enumerate(tiles):
            if k == len(tiles) - 1:
                # params transfer right before the last hidden tile: they are
                # only needed by the epilogue
                dma_chain.append(nc.sync.dma_start(out=cf_sb[:], in_=cf_d[:]))
                dma_chain.append(nc.sync.dma_start(out=cb_sb[:], in_=cb_d[:]))
            ht = hidp.tile([128, TS // 128, H], BF16)
            dma_chain.append(
                nc.sync.dma_start(
                    out=ht[:],
                    in_=hid_d[b, t * TS:(t + 1) * TS, :].rearrange(
                        "(c p) h -> p c h", p=128
                    ),
                )
            )
            for c in range(TS // 128):
                n = b * (S // 128) + t * (TS // 128) + c
                lw = wm_sb[:, n * 8:(n + 1) * 8]
                for j in range(2):
                    mm = nc.tensor.matmul(
                        pooled_ps[:, j * 512:(j + 1) * 512],
                        lhsT=lw,
                        rhs=ht[:, c, j * 512:(j + 1) * 512],
                        start=(n == 0),
                        stop=(n == NK - 1),
                    )
                    if first_mm is None:
                        first_mm = mm
                    last_mm = mm

        for k in range(1, len(dma_chain)):
            add_dep_helper(
                dma_chain[k].ins, dma_chain[k - 1].ins, sync=False,
                reason="serialize sync-ring DMAs",
            )
        add_dep_helper(first_mm.ins, wm_abs.ins, sync=False, reason="absorb wm dma wait")

        # absorbers/touches for epilogue consts; PE ones anchored after the
        # main loop (PE is idle during the DVE epilogue prologue).
        cb_abs = absorb(cb_sb)
        cf_abs = absorb(cf_sb)
        for a in (cb_abs, cf_abs):
            add_dep_helper(a.ins, last_mm.ins, sync=False, reason="absorber after main loop")
        tv_cf = work.tile([1, 1], F32)
        t_cf = nc.vector.tensor_copy(tv_cf[0:1, 0:1], cf_sb[0:1, 0:1])
        ta_cf = work.tile([128, 1], F32)
        a_cf = nc.scalar.copy(out=ta_cf[:, 0:1], in_=cf_sb[:, 0:1])

        # ---- epilogue ----
        # Compute-engine APs must start at partition 0/32/64/96, so all
        # cross-row arithmetic happens after transposing to the free dim.
        # P4 rows: 0-1 pooled(s0,s1), 2-3 first, 4-5 second, 6-7 ending
        # The 1/count scaling runs on ACT (Copy with per-partition scale)
        # while DVE computes the LayerNorm stats straight from raw PSUM:
        # mu' = mu_raw*inv, rstd' = 1/sqrt(var_raw*inv^2 + eps), and
        # xn = (raw - mu_raw) * (inv * rstd').
        P4 = work.tile([8, H], F32)
        p4op = nc.scalar.activation(
            out=P4[:], in_=pooled_ps[:],
            func=mybir.ActivationFunctionType.Copy, bias=0.0, scale=invc_v,
        )
        add_dep_helper(p4op.ins, a_cf.ins, sync=False, reason="cf act touch first")

        stats = work.tile([2, 2, 6], F32)
        nc.vector.bn_stats(out=stats[:, 0, :], in_=pooled_ps[0:2, 0:512])
        nc.vector.bn_stats(out=stats[:, 1, :], in_=pooled_ps[0:2, 512:1024])
        mv = work.tile([2, 2], F32)
        bnop = nc.vector.bn_aggr(out=mv[:], in_=stats[:])
        add_dep_helper(bnop.ins, t_cf.ins, sync=False, reason="cf touch first")
        iv2 = work.tile([2, 1], F32)
        nc.vector.tensor_mul(iv2[:], invc_v[0:2, :], invc_v[0:2, :])
        vsc = work.tile([2, 1], F32)
        nc.vector.tensor_mul(vsc[:], mv[:, 1:2], iv2[:])
        eps_sb = work.tile([2, 1], F32)
        nc.vector.memset(eps_sb[:], LN_EPS)
        rstd = work.tile([2, 1], F32)
        sqop = nc.scalar.activation(
            out=rstd[:], in_=vsc[:],
            func=mybir.ActivationFunctionType.Sqrt, bias=eps_sb[:], scale=1.0,
        )
        # re-warm the Erf table right after the (sole) Sqrt use so the later
        # Erf activations don't pay the table load on the critical chain
        erf_rewarm = nc.scalar.activation(
            out=ws_out[:], in_=ws_in[:],
            func=mybir.ActivationFunctionType.Erf, bias=ws_b[:],
        )
        add_dep_helper(erf_rewarm.ins, sqop.ins, sync=False, reason="erf rewarm after sqrt")
        nc.vector.reciprocal(rstd[:], rstd[:])
        mu2 = work.tile([2, 1], F32)
        nc.vector.tensor_mul(mu2[:], mv[:, 0:1], invc_v[0:2, :])
        xn = work.tile([2, H], F32)
        nc.vector.tensor_scalar(
            out=xn[:], in0=P4[0:2, :], scalar1=mu2[:], scalar2=rstd[:],
            op0=mybir.AluOpType.subtract, op1=mybir.AluOpType.mult,
        )

        # XTR[:, 10c + r]: r in 0..8 = P4 row r, r in 8..10 = xn row r-8,
        # for H positions c*128..(c+1)*128 on partitions.  The P4 transposes
        # and their cast run first so the esc/res/end heads are unblocked
        # before the LayerNorm (xn) path resolves.
        xtr_ps = pssm.tile([128, 80], F32)
        xtr_v = xtr_ps[:].rearrange("p (c r) -> p c r", r=10)
        XTR = work.tile([128, 8, 10], BF16)
        first_tr = None
        for cc in range(8):
            tr = nc.tensor.transpose(
                out=xtr_ps[:, cc * 10:cc * 10 + 8],
                in_=P4[:, cc * 128:(cc + 1) * 128],
                identity=id8_v,
            )
            if first_tr is None:
                first_tr = tr
                for a in (cb_abs, cf_abs):
                    add_dep_helper(first_tr.ins, a.ins, sync=False, reason="absorbers before epilogue")
        nc.vector.tensor_copy(XTR[:, :, 0:8], xtr_v[:, :, 0:8])
        for cc in range(8):
            nc.tensor.transpose(
                out=xtr_ps[:, cc * 10 + 8:cc * 10 + 10],
                in_=xn[:, cc * 128:(cc + 1) * 128],
                identity=i2_v,
            )
        nc.vector.tensor_copy(XTR[:, :, 8:10], xtr_v[:, :, 8:10])

        # head inputs on the free dim: esc = relu(second-first), res = relu(-d)
        dT = work.tile([128, 8, 2], BF16)
        nc.vector.tensor_sub(dT[:], XTR[:, :, 4:6], XTR[:, :, 2:4])
        escT = work.tile([128, 8, 2], BF16)
        nc.vector.tensor_scalar_max(out=escT[:], in0=dT[:], scalar1=0.0)
        resT = work.tile([128, 8, 2], BF16)
        nc.vector.tensor_scalar(
            out=resT[:], in0=dT[:], scalar1=-1.0, scalar2=0.0,
            op0=mybir.AluOpType.mult, op1=mybir.AluOpType.max,
        )

        def head_rhs(h, cc):
            if h == 0:
                return escT[:, cc, :]
            if h == 1:
                return resT[:, cc, :]
            if h == 2:
                return XTR[:, cc, 6:8]
            return XTR[:, cc, 8:10]

        # head first layers: h1[:, 2h+j] = b1_h + w1_h.T @ x_{h,j}
        h1_ps = pssm.tile([128, 8], F32)
        for h in range(4):
            nc.tensor.matmul(
                h1_ps[:, 2 * h:2 * h + 2], lhsT=b1r_v(h), rhs=ones_v,
                start=True, stop=False,
            )
            for cc in range(8):
                nc.tensor.matmul(
                    h1_ps[:, 2 * h:2 * h + 2],
                    lhsT=w1_v(h, cc),
                    rhs=head_rhs(h, cc),
                    start=False,
                    stop=(cc == 7),
                )
        # exact GELU: y=erf(z/sqrt(2)), g=z+z*y  (0.5 folded into mh weights)
        y1 = work.tile([128, 8], F32)
        y1op = nc.scalar.activation(
            out=y1[:], in_=h1_ps[:],
            func=mybir.ActivationFunctionType.Erf, bias=zero_v, scale=RS2,
        )
        add_dep_helper(y1op.ins, erf_rewarm.ins, sync=False, reason="erf rewarmed first")
        t1 = work.tile([128, 8], F32)
        nc.vector.tensor_mul(t1[:], h1_ps[:], y1[:])
        g1 = work.tile([128, 8], BF16)
        nc.vector.tensor_add(g1[:], h1_ps[:], t1[:])

        # fc1[:, 2m+j] = fb1 + fc_w1.T @ pooled_j + sum_h mh_h.T @ g1_{h,j}
        fc1_ps = pssm.tile([128, 4], F32)
        for m in range(2):
            sl = slice(2 * m, 2 * m + 2)
            nc.tensor.matmul(
                fc1_ps[:, sl], lhsT=fb1r_v(m), rhs=ones_v,
                start=True, stop=False,
            )
            for cc in range(8):
                nc.tensor.matmul(
                    fc1_ps[:, sl],
                    lhsT=fw1_v(cc, m),
                    rhs=XTR[:, cc, 8:10],
                    start=False,
                    stop=False,
                )
            for h in range(4):
                nc.tensor.matmul(
                    fc1_ps[:, sl],
                    lhsT=mh_v(h, m),
                    rhs=g1[:, 2 * h:2 * h + 2],
                    start=False,
                    stop=(h == 3),
                )
        y2 = work.tile([128, 4], F32)
        nc.scalar.activation(
            out=y2[:], in_=fc1_ps[:],
            func=mybir.ActivationFunctionType.Erf, bias=zero_v, scale=RS2,
        )
        t2 = work.tile([128, 4], F32)
        nc.vector.tensor_mul(t2[:], fc1_ps[:], y2[:])
        g2 = work.tile([128, 4], BF16)
        nc.vector.tensor_add(g2[:], fc1_ps[:], t2[:])

        out_ps = pssm.tile([5, 2], F32)
        nc.tensor.matmul(out_ps[:], lhsT=fb2r_v, rhs=ones_v, start=True, stop=False)
        for m in range(2):
            nc.tensor.matmul(
                out_ps[:],
                lhsT=fw2_v(m),
                rhs=g2[:, 2 * m:2 * m + 2],
                start=False,
                stop=(m == 1),
            )
        out_sb = work.tile([5, 2], F32)
        nc.vector.tensor_copy(out_sb[:], out_ps[:])
        nc.gpsimd.dma_start(out=out_d[:, :], in_=out_sb[:])

    nc.compile()
    return nc


def _pack_k_major(w, k, m):
    """[K, M] -> [128, (K//128)*M] with lhsT chunk c at cols [c*M, (c+1)*M)."""
    return np.ascontiguousarray(
        w.reshape(k // 128, 128, m).transpose(1, 0, 2).reshape(128, (k // 128) * m)
    ).astype(np.float32)


def _host_prep(inputs):
    """Build all per-core in_maps from the full inputs."""
    f32 = np.float32
    bf16 = ml_dtypes.bfloat16
    am = np.asarray(inputs["attention_mask"])
    hid = np.asarray(inputs["hidden"], dtype=f32)

    m_full = am.astype(f32)                      # [B, S]
    L = am.astype(np.int64).sum(1)               # [B]
    pos = np.arange(S)[None, :]
    mid = (L // 2)[:, None]
    Lb = L[:, None]
    st = np.maximum(1, L - 64)[:, None]
    fm = ((pos >= 1) & (pos < mid)).astype(f32)
    sm = ((pos >= mid) & (pos < Lb - 1)).astype(f32)
    em = ((pos >= st) & (pos < Lb - 1)).astype(f32)
    masks = [m_full, fm, sm, em]                 # type order: pooled,first,second,ending
    invs = [
        (1.0 / np.maximum(mk.sum(1, dtype=np.float64), EPS)).astype(f32)
        for mk in masks
    ]

    ln_g = np.asarray(inputs["ln_g"], np.float64)
    ln_b = np.asarray(inputs["ln_b"], np.float64)

    fc_w1 = np.asarray(inputs["fc_w1"], f32)     # [H+4, 256]
    fc_b1 = np.asarray(inputs["fc_b1"], f32)
    fc_w2 = np.asarray(inputs["fc_w2"], f32)     # [256, 5]
    fc_b2 = np.asarray(inputs["fc_b2"], f32)

    # packed const blocks
    cf = np.zeros((128, CF_COLS), f32)
    cf[0:8, CF_ID8:CF_ID8 + 8] = np.eye(8, dtype=f32)
    cb = np.zeros((128, CB_COLS), bf16)
    cb[0, CB_FB2R:CB_FB2R + 5] = fc_b2.astype(bf16)
    cb[0, CB_ONES:CB_ONES + 2] = np.ones(2, bf16)

    fb1_eff = fc_b1.astype(np.float64) + ln_b @ fc_w1[:H].astype(np.float64)
    for h, name in enumerate(HEADS):
        w1 = np.asarray(inputs[f"{name}_w1"], f32).astype(np.float64)  # [H, 128]
        b1 = np.asarray(inputs[f"{name}_b1"], f32).astype(np.float64)  # [128]
        w2 = np.asarray(inputs[f"{name}_w2"], f32)   # [128, 1]
        b2 = np.asarray(inputs[f"{name}_b2"], f32)   # [1]
        if name == "thr":
            # fold the LayerNorm affine into the thr head input weights
            b1 = b1 + ln_b @ w1
            w1 = ln_g[:, None] * w1
        cb[:, CB_W1 + 1024 * h:CB_W1 + 1024 * (h + 1)] = _pack_k_major(
            w1.astype(f32), H, 128
        ).astype(bf16)
        cb[0, CB_B1R + 128 * h:CB_B1R + 128 * (h + 1)] = b1.astype(bf16)
        cb[:, CB_MH + 256 * h:CB_MH + 256 * (h + 1)] = np.ascontiguousarray(
            0.5 * w2[:, 0][:, None] * fc_w1[H + h, :][None, :]
        ).astype(bf16)
        fb1_eff = fb1_eff + b2[0] * fc_w1[H + h, :].astype(np.float64)

    fw1_folded = (ln_g[:, None] * fc_w1[:H].astype(np.float64)).astype(f32)
    cb[:, CB_FW1:CB_FW1 + 2048] = _pack_k_major(fw1_folded, H, 256).astype(bf16)
    cb[:, CB_FW2:CB_FW2 + 10] = _pack_k_major(0.5 * fc_w2, 256, 5).astype(bf16)
    fb1_eff = fb1_eff.astype(f32)
    cb[0, CB_FB1R:CB_FB1R + 128] = fb1_eff[0:128].astype(bf16)
    cb[0, CB_FB1R + 128:CB_FB1R + 256] = fb1_eff[128:256].astype(bf16)

    in_maps = []
    for i in range(NCORES):
        msk = np.zeros((BPC, S // 128, 128, 8), f32)
        cf_i = cf.copy()
        for b in range(BPC):
            gb = BPC * i + b
            for ty in range(4):
                msk[b, :, :, 2 * ty + b] = masks[ty][gb].reshape(S // 128, 128)
                cf_i[2 * ty + b, CF_INVC] = invs[ty][gb]
        wm = np.ascontiguousarray(
            msk.reshape(NK, 128, 8).transpose(1, 0, 2).reshape(128, NK * 8)
        ).astype(bf16)
        in_maps.append(
            dict(
                hid=np.ascontiguousarray(hid[BPC * i:BPC * (i + 1)]).astype(bf16),
                wm=wm,
                cb=cb,
                cf=cf_i,
            )
        )
    return in_maps


def kernel(**inputs):
    if "nc" not in _NC_CACHE:
        _NC_CACHE["nc"] = _build_nc()
    nc = _NC_CACHE["nc"]
    in_maps = _host_prep(inputs)
    res = run_bass_kernel_spmd(nc, in_maps, core_ids=list(range(NCORES)))
    out = np.empty((B, 5), np.float32)
    for i in range(NCORES):
        out[BPC * i:BPC * (i + 1)] = res.results[i]["out"].T
    return out


# revision 43
# speedup vs baseline: 2.5757x; 1.1142x over previous
"""Trainium2 Bass kernel for nn_DirectionalMultiHeadClassifier.

Data-parallel over 8 NeuronCores: each core handles 2 of the 16 samples.

Math per sample (mirrors the reference):
  - 4 masked means over S of hidden [S,H]: full attention_mask, and three
    position-range masks derived from L = mask.sum() (first/second/ending).
    Computed on-device as one PSUM-accumulated matmul:
        pooled4[8, H] += W_chunk[128, 8].T @ hidden_chunk[128, H]
    where W is a host-built 0/1 mask matrix (4 mask types x 2 samples) and
    the 1/count normalization is applied afterwards.
  - LayerNorm on the full-mask pooled vector; ln_g/ln_b are folded on the
    host into every consumer of the normalized vector (thr head w1/b1 and
    the fc pooled-part weights/bias), so the device only normalizes.
  - 4 small MLP heads (H->128 -> exact GELU -> 128->1). The scalar head
    outputs only feed the final classifier's last 4 input features, so the
    128->1 layer is folded into the classifier on the host:
        fc1 += gelu_h @ (0.5 * w2_h outer fc_w1[1024+h, :])
        fc_b1_eff = fc_b1 + sum_h b2_h * fc_w1[1024+h, :]
  - Final classifier (1028->256 -> exact GELU -> 256->5).
  Exact GELU is computed as 0.5*z*(1+erf(z/sqrt(2))) with the 0.5 folded
  into the following layer's weights.  Every linear bias is applied as a
  K=1 rank-1 matmul (bias_row outer ones) accumulated into PSUM, so the
  GELU needs just one Erf activation per layer.

Compute dtype: hidden/masks/weights stream through the PE in bf16 (masks
are exact 0/1 in bf16); all accumulation is f32 in PSUM.
"""

import ml_dtypes
import numpy as np

import concourse.bass as bass
import concourse.tile as tile
from bass_rust import add_dep_helper
from concourse import bacc, mybir
from concourse.bass_utils import run_bass_kernel_spmd

B, S, H = 16, 2048, 1024
NCORES = 8
BPC = B // NCORES          # samples per core
NK = BPC * (S // 128)      # 128-row contraction chunks per core
TS = 512                   # S rows per hidden DMA tile (1 MiB bf16)
NT = S // TS               # DMA tiles per sample
RS2 = 0.7071067811865476   # 1/sqrt(2)
LN_EPS = 1e-5
EPS = 1e-9
F32 = mybir.dt.float32
BF16 = mybir.dt.bfloat16
HEADS = ["esc", "res", "end", "thr"]

# packed bf16 const-block column offsets
CB_W1 = 0                  # 4 x [128, 1024]
CB_MH = 4096               # 4 x [128, 256]
CB_FW1 = 5120              # [128, 2048]
CB_FW2 = 7168              # [128, 10]
CB_B1R = 7178              # 4 x [1, 128] bias rows (row 0)
CB_FB1R = 7690             # 2 x [1, 128] fc bias rows (row 0)
CB_FB2R = 7946             # [1, 5] out bias row (row 0)
CB_ONES = 7951             # [1, 2] ones (row 0)
CB_COLS = 7953
# packed f32 const-block column offsets
CF_INVC = 0                # [8, 1]
CF_ID8 = 1                 # [8, 8]
CF_ZERO = 9                # [128, 1] zeros (activation bias)
CF_COLS = 10

_NC_CACHE = {}


def _build_nc():
    """Build the per-core Bass program (identical on all 8 cores)."""
    from contextlib import ExitStack

    nc = bacc.Bacc(
        "TRN2", target_bir_lowering=False, debug=False, num_devices=NCORES
    )
    dp = nc.declare_dram_parameter
    hid_d = dp("hid", [BPC, S, H], BF16, isOutput=False)
    wm_d = dp("wm", [128, NK * 8], BF16, isOutput=False)
    cb_d = dp("cb", [128, CB_COLS], BF16, isOutput=False)
    cf_d = dp("cf", [128, CF_COLS], F32, isOutput=False)
    out_d = dp("out", [5, BPC], F32, isOutput=True)

    with tile.TileContext(nc) as tc, ExitStack() as ctx:
        const = ctx.enter_context(tc.tile_pool(name="const", bufs=1))
        hidp = ctx.enter_context(tc.tile_pool(name="hidp", bufs=BPC * NT))
        work = ctx.enter_context(tc.tile_pool(name="work", bufs=1))
        psmain = ctx.enter_context(tc.tile_pool(name="psmain", bufs=1, space="PSUM"))
        pssm = ctx.enter_context(tc.tile_pool(name="pssm", bufs=1, space="PSUM"))

        # ACT table warm-up: touch the activation functions used later so the
        # ~1.3us/table loads overlap the initial DMAs instead of serializing
        # into the epilogue.
        ws_in = work.tile([1, 1], F32)
        ws_b = work.tile([1, 1], F32)
        ws_out = work.tile([1, 1], F32)
        nc.vector.memset(ws_in[:], 0.0)
        nc.vector.memset(ws_b[:], 0.0)
        for fn in (
            mybir.ActivationFunctionType.Erf,
            mybir.ActivationFunctionType.Sqrt,
        ):
            nc.scalar.activation(out=ws_out[:], in_=ws_in[:], func=fn, bias=ws_b[:])

        # All large DMAs go on the single sync HWDGE ring, explicitly chained
        # so they transfer strictly in this order: wm, tile1..3, consts,
        # tile4.  Sequential transfers hand each tile over ASAP (concurrent
        # round-robin would delay the FIRST tile by 4x) and the params arrive
        # right before the epilogue needs them.
        wm_sb = const.tile([128, NK * 8], BF16, name="c_wm", tag="c_wm")
        cb_sb = const.tile([128, CB_COLS], BF16, name="c_cb", tag="c_cb")
        cf_sb = const.tile([128, CF_COLS], F32, name="c_cf", tag="c_cf")
        dma_chain = [
            nc.sync.dma_start(out=cf_sb[:], in_=cf_d[:]),
            nc.sync.dma_start(out=wm_sb[:], in_=wm_d[:]),
        ]

        # const views
        invc_v = cf_sb[0:8, CF_INVC:CF_INVC + 1]
        id8_v = cf_sb[0:8, CF_ID8:CF_ID8 + 8]
        i2_v = cf_sb[0:2, CF_ID8:CF_ID8 + 2]
        zero_v = cf_sb[:, CF_ZERO:CF_ZERO + 1]
        w1_v = lambda h, c: cb_sb[:, CB_W1 + 1024 * h + 128 * c:CB_W1 + 1024 * h + 128 * (c + 1)]
        mh_v = lambda h, m: cb_sb[:, CB_MH + 256 * h + 128 * m:CB_MH + 256 * h + 128 * (m + 1)]
        fw1_v = lambda c, m: cb_sb[:, CB_FW1 + 256 * c + 128 * m:CB_FW1 + 256 * c + 128 * (m + 1)]
        fw2_v = lambda m: cb_sb[:, CB_FW2 + 5 * m:CB_FW2 + 5 * (m + 1)]
        b1r_v = lambda h: cb_sb[0:1, CB_B1R + 128 * h:CB_B1R + 128 * (h + 1)]
        fb1r_v = lambda m: cb_sb[0:1, CB_FB1R + 128 * m:CB_FB1R + 128 * (m + 1)]
        fb2r_v = cb_sb[0:1, CB_FB2R:CB_FB2R + 5]
        ones_v = cb_sb[0:1, CB_ONES:CB_ONES + 2]

        # Wait-absorbers: every engine instruction carries at most ONE
        # semaphore wait in this walrus build, so consume each const DMA's
        # completion once per reading engine; real consumers then only wait
        # on their data inputs.
        scr_ps = pssm.tile([8, 8], F32)

        def absorb(csb):
            return nc.tensor.matmul(
                scr_ps[:, :], lhsT=csb[:, 0:8], rhs=csb[:, 0:8],
                start=True, stop=True,
            )

        # PE warm-up: the HAM clock gate defaults to 1.2 GHz and needs ~3.4us
        # of sustained activity to unthrottle.  Run junk matmuls during the
        # initial DMA wait so the real loop starts (and stays) at 2.4 GHz.
        warm_in = work.tile([128, 256], BF16)
        nc.vector.memset(warm_in[:], 0.0)
        warm_ps = pssm.tile([8, 512], F32)
        warm_last = None
        for _ in range(72):
            warm_last = nc.tensor.matmul(
                warm_ps[:, 0:256], lhsT=warm_in[:, 0:8], rhs=warm_in[:, 0:256],
                start=True, stop=True,
            )

        wm_abs = absorb(wm_sb)
        add_dep_helper(wm_abs.ins, warm_last.ins, sync=False, reason="warmup before wm absorber")

        # ---- main loop: pooled4[j, h] = sum_s wm[s, j] * hidden[s, h] ----
        pooled_ps = psmain.tile([8, H], F32)
        first_mm = None
        last_mm = None
        tiles = [(b, t) for b in range(BPC) for t in range(NT)]
        for k, (b, t) in enumerate(tiles):
            ht = hidp.tile([128, TS // 128, H], BF16)
            dma_chain.append(
                nc.sync.dma_start(
                    out=ht[:],
                    in_=hid_d[b, t * TS:(t + 1) * TS, :].rearrange(
                        "(c p) h -> p c h", p=128
                    ),
                )
            )
            for c in range(TS // 128):
                n = b * (S // 128) + t * (TS // 128) + c
                lw = wm_sb[:, n * 8:(n + 1) * 8]
                for j in range(2):
                    mm = nc.tensor.matmul(
                        pooled_ps[:, j * 512:(j + 1) * 512],
                        lhsT=lw,
                        rhs=ht[:, c, j * 512:(j + 1) * 512],
                        start=(n == 0),
                        stop=(n == NK - 1),
                    )
                    if first_mm is None:
                        first_mm = mm
                    last_mm = mm

        # the epilogue weight block transfers LAST: the whole pre-head part of
        # the epilogue (LayerNorm, transposes, relu inputs) doesn't need it,
        # so it streams in parallel with those stages.
        dma_chain.append(nc.sync.dma_start(out=cb_sb[:], in_=cb_d[:]))
        for k in range(1, len(dma_chain)):
            add_dep_helper(
                dma_chain[k].ins, dma_chain[k - 1].ins, sync=False,
                reason="serialize sync-ring DMAs",
            )
        add_dep_helper(first_mm.ins, wm_abs.ins, sync=False, reason="absorb wm dma wait")

        # absorbers/touches for epilogue consts; cf is tiny and arrives first
        # (absorb before the main loop), cb arrives last (absorb after it).
        cf_abs = absorb(cf_sb)
        add_dep_helper(cf_abs.ins, wm_abs.ins, sync=False, reason="cf absorber after warmup")
        add_dep_helper(first_mm.ins, cf_abs.ins, sync=False, reason="cf absorbed before main loop")
        cb_abs = absorb(cb_sb)
        add_dep_helper(cb_abs.ins, last_mm.ins, sync=False, reason="absorber after main loop")
        tv_cf = work.tile([1, 1], F32)
        t_cf = nc.vector.tensor_copy(tv_cf[0:1, 0:1], cf_sb[0:1, 0:1])
        ta_cf = work.tile([128, 1], F32)
        a_cf = nc.scalar.copy(out=ta_cf[:, 0:1], in_=cf_sb[:, 0:1])

        # ---- epilogue ----
        # Compute-engine APs must start at partition 0/32/64/96, so all
        # cross-row arithmetic happens after transposing to the free dim.
        # P4 rows: 0-1 pooled(s0,s1), 2-3 first, 4-5 second, 6-7 ending
        # The 1/count scaling runs on ACT (Copy with per-partition scale)
        # while DVE computes the LayerNorm stats straight from raw PSUM:
        # mu' = mu_raw*inv, rstd' = 1/sqrt(var_raw*inv^2 + eps), and
        # xn = (raw - mu_raw) * (inv * rstd').
        P4 = work.tile([8, H], F32)
        p4op = nc.scalar.activation(
            out=P4[:], in_=pooled_ps[:],
            func=mybir.ActivationFunctionType.Copy, bias=0.0, scale=invc_v,
        )
        add_dep_helper(p4op.ins, a_cf.ins, sync=False, reason="cf act touch first")

        stats = work.tile([2, 2, 6], F32)
        nc.vector.bn_stats(out=stats[:, 0, :], in_=pooled_ps[0:2, 0:512])
        nc.vector.bn_stats(out=stats[:, 1, :], in_=pooled_ps[0:2, 512:1024])
        mv = work.tile([2, 2], F32)
        bnop = nc.vector.bn_aggr(out=mv[:], in_=stats[:])
        add_dep_helper(bnop.ins, t_cf.ins, sync=False, reason="cf touch first")
        iv2 = work.tile([2, 1], F32)
        nc.vector.tensor_mul(iv2[:], invc_v[0:2, :], invc_v[0:2, :])
        vsc = work.tile([2, 1], F32)
        nc.vector.tensor_mul(vsc[:], mv[:, 1:2], iv2[:])
        eps_sb = work.tile([2, 1], F32)
        nc.vector.memset(eps_sb[:], LN_EPS)
        rstd = work.tile([2, 1], F32)
        sqop = nc.scalar.activation(
            out=rstd[:], in_=vsc[:],
            func=mybir.ActivationFunctionType.Sqrt, bias=eps_sb[:], scale=1.0,
        )
        # re-warm the Erf table right after the (sole) Sqrt use so the later
        # Erf activations don't pay the table load on the critical chain
        erf_rewarm = nc.scalar.activation(
            out=ws_out[:], in_=ws_in[:],
            func=mybir.ActivationFunctionType.Erf, bias=ws_b[:],
        )
        add_dep_helper(erf_rewarm.ins, sqop.ins, sync=False, reason="erf rewarm after sqrt")
        nc.vector.reciprocal(rstd[:], rstd[:])
        mu2 = work.tile([2, 1], F32)
        nc.vector.tensor_mul(mu2[:], mv[:, 0:1], invc_v[0:2, :])
        xn = work.tile([2, H], F32)
        nc.vector.tensor_scalar(
            out=xn[:], in0=P4[0:2, :], scalar1=mu2[:], scalar2=rstd[:],
            op0=mybir.AluOpType.subtract, op1=mybir.AluOpType.mult,
        )

        # XTR[:, 10c + r]: r in 0..8 = P4 row r, r in 8..10 = xn row r-8,
        # for H positions c*128..(c+1)*128 on partitions.  The P4 transposes
        # and their cast run first so the esc/res/end heads are unblocked
        # before the LayerNorm (xn) path resolves.
        xtr_ps = pssm.tile([128, 80], F32)
        xtr_v = xtr_ps[:].rearrange("p (c r) -> p c r", r=10)
        XTR = work.tile([128, 8, 10], BF16)
        first_tr = None
        for cc in range(8):
            tr = nc.tensor.transpose(
                out=xtr_ps[:, cc * 10:cc * 10 + 8],
                in_=P4[:, cc * 128:(cc + 1) * 128],
                identity=id8_v,
            )
            if first_tr is None:
                first_tr = tr
                add_dep_helper(first_tr.ins, cf_abs.ins, sync=False, reason="cf absorbed before transposes")
        nc.vector.tensor_copy(XTR[:, :, 0:8], xtr_v[:, :, 0:8])
        for cc in range(8):
            nc.tensor.transpose(
                out=xtr_ps[:, cc * 10 + 8:cc * 10 + 10],
                in_=xn[:, cc * 128:(cc + 1) * 128],
                identity=i2_v,
            )
        nc.vector.tensor_copy(XTR[:, :, 8:10], xtr_v[:, :, 8:10])

        # head inputs on the free dim: esc = relu(second-first), res = relu(-d)
        dT = work.tile([128, 8, 2], BF16)
        nc.vector.tensor_sub(dT[:], XTR[:, :, 4:6], XTR[:, :, 2:4])
        escT = work.tile([128, 8, 2], BF16)
        nc.vector.tensor_scalar_max(out=escT[:], in0=dT[:], scalar1=0.0)
        resT = work.tile([128, 8, 2], BF16)
        nc.vector.tensor_scalar(
            out=resT[:], in0=dT[:], scalar1=-1.0, scalar2=0.0,
            op0=mybir.AluOpType.mult, op1=mybir.AluOpType.max,
        )

        def head_rhs(h, cc):
            if h == 0:
                return escT[:, cc, :]
            if h == 1:
                return resT[:, cc, :]
            if h == 2:
                return XTR[:, cc, 6:8]
            return XTR[:, cc, 8:10]

        # head first layers: h1[:, 2h+j] = b1_h + w1_h.T @ x_{h,j}
        h1_ps = pssm.tile([128, 8], F32)
        for h in range(4):
            bmm = nc.tensor.matmul(
                h1_ps[:, 2 * h:2 * h + 2], lhsT=b1r_v(h), rhs=ones_v,
                start=True, stop=False,
            )
            if h == 0:
                add_dep_helper(bmm.ins, cb_abs.ins, sync=False, reason="cb absorbed before heads")
            for cc in range(8):
                nc.tensor.matmul(
                    h1_ps[:, 2 * h:2 * h + 2],
                    lhsT=w1_v(h, cc),
                    rhs=head_rhs(h, cc),
                    start=False,
                    stop=(cc == 7),
                )
        # exact GELU: y=erf(z/sqrt(2)), g=z+z*y  (0.5 folded into mh weights)
        y1 = work.tile([128, 8], F32)
        y1op = nc.scalar.activation(
            out=y1[:], in_=h1_ps[:],
            func=mybir.ActivationFunctionType.Erf, bias=zero_v, scale=RS2,
        )
        add_dep_helper(y1op.ins, erf_rewarm.ins, sync=False, reason="erf rewarmed first")
        t1 = work.tile([128, 8], F32)
        nc.vector.tensor_mul(t1[:], h1_ps[:], y1[:])
        g1 = work.tile([128, 8], BF16)
        nc.vector.tensor_add(g1[:], h1_ps[:], t1[:])

        # fc1[:, 2m+j] = fb1 + fc_w1.T @ pooled_j + sum_h mh_h.T @ g1_{h,j}
        fc1_ps = pssm.tile([128, 4], F32)
        for m in range(2):
            sl = slice(2 * m, 2 * m + 2)
            nc.tensor.matmul(
                fc1_ps[:, sl], lhsT=fb1r_v(m), rhs=ones_v,
                start=True, stop=False,
            )
            for cc in range(8):
                nc.tensor.matmul(
                    fc1_ps[:, sl],
                    lhsT=fw1_v(cc, m),
                    rhs=XTR[:, cc, 8:10],
                    start=False,
                    stop=False,
                )
            for h in range(4):
                nc.tensor.matmul(
                    fc1_ps[:, sl],
                    lhsT=mh_v(h, m),
                    rhs=g1[:, 2 * h:2 * h + 2],
                    start=False,
                    stop=(h == 3),
                )
        y2 = work.tile([128, 4], F32)
        nc.scalar.activation(
            out=y2[:], in_=fc1_ps[:],
            func=mybir.ActivationFunctionType.Erf, bias=zero_v, scale=RS2,
        )
        t2 = work.tile([128, 4], F32)
        nc.vector.tensor_mul(t2[:], fc1_ps[:], y2[:])
        g2 = work.tile([128, 4], BF16)
        nc.vector.tensor_add(g2[:], fc1_ps[:], t2[:])

        out_ps = pssm.tile([5, 2], F32)
        nc.tensor.matmul(out_ps[:], lhsT=fb2r_v, rhs=ones_v, start=True, stop=False)
        for m in range(2):
            nc.tensor.matmul(
                out_ps[:],
                lhsT=fw2_v(m),
                rhs=g2[:, 2 * m:2 * m + 2],
                start=False,
                stop=(m == 1),
            )
        out_sb = work.tile([5, 2], F32)
        nc.vector.tensor_copy(out_sb[:], out_ps[:])
        nc.gpsimd.dma_start(out=out_d[:, :], in_=out_sb[:])

    nc.compile()
    return nc


def _pack_k_major(w, k, m):
    """[K, M] -> [128, (K//128)*M] with lhsT chunk c at cols [c*M, (c+1)*M)."""
    return np.ascontiguousarray(
        w.reshape(k // 128, 128, m).transpose(1, 0, 2).reshape(128, (k // 128) * m)
    ).astype(np.float32)


def _host_prep(inputs):
    """Build all per-core in_maps from the full inputs."""
    f32 = np.float32
    bf16 = ml_dtypes.bfloat16
    am = np.asarray(inputs["attention_mask"])
    hid = np.asarray(inputs["hidden"], dtype=f32)

    m_full = am.astype(f32)                      # [B, S]
    L = am.astype(np.int64).sum(1)               # [B]
    pos = np.arange(S)[None, :]
    mid = (L // 2)[:, None]
    Lb = L[:, None]
    st = np.maximum(1, L - 64)[:, None]
    fm = ((pos >= 1) & (pos < mid)).astype(f32)
    sm = ((pos >= mid) & (pos < Lb - 1)).astype(f32)
    em = ((pos >= st) & (pos < Lb - 1)).astype(f32)
    masks = [m_full, fm, sm, em]                 # type order: pooled,first,second,ending
    invs = [
        (1.0 / np.maximum(mk.sum(1, dtype=np.float64), EPS)).astype(f32)
        for mk in masks
    ]

    ln_g = np.asarray(inputs["ln_g"], np.float64)
    ln_b = np.asarray(inputs["ln_b"], np.float64)

    fc_w1 = np.asarray(inputs["fc_w1"], f32)     # [H+4, 256]
    fc_b1 = np.asarray(inputs["fc_b1"], f32)
    fc_w2 = np.asarray(inputs["fc_w2"], f32)     # [256, 5]
    fc_b2 = np.asarray(inputs["fc_b2"], f32)

    # packed const blocks
    cf = np.zeros((128, CF_COLS), f32)
    cf[0:8, CF_ID8:CF_ID8 + 8] = np.eye(8, dtype=f32)
    cb = np.zeros((128, CB_COLS), bf16)
    cb[0, CB_FB2R:CB_FB2R + 5] = fc_b2.astype(bf16)
    cb[0, CB_ONES:CB_ONES + 2] = np.ones(2, bf16)

    fb1_eff = fc_b1.astype(np.float64) + ln_b @ fc_w1[:H].astype(np.float64)
    for h, name in enumerate(HEADS):
        w1 = np.asarray(inputs[f"{name}_w1"], f32).astype(np.float64)  # [H, 128]
        b1 = np.asarray(inputs[f"{name}_b1"], f32).astype(np.float64)  # [128]
        w2 = np.asarray(inputs[f"{name}_w2"], f32)   # [128, 1]
        b2 = np.asarray(inputs[f"{name}_b2"], f32)   # [1]
        if name == "thr":
            # fold the LayerNorm affine into the thr head input weights
            b1 = b1 + ln_b @ w1
            w1 = ln_g[:, None] * w1
        cb[:, CB_W1 + 1024 * h:CB_W1 + 1024 * (h + 1)] = _pack_k_major(
            w1.astype(f32), H, 128
        ).astype(bf16)
        cb[0, CB_B1R + 128 * h:CB_B1R + 128 * (h + 1)] = b1.astype(bf16)
        cb[:, CB_MH + 256 * h:CB_MH + 256 * (h + 1)] = np.ascontiguousarray(
            0.5 * w2[:, 0][:, None] * fc_w1[H + h, :][None, :]
        ).astype(bf16)
        fb1_eff = fb1_eff + b2[0] * fc_w1[H + h, :].astype(np.float64)

    fw1_folded = (ln_g[:, None] * fc_w1[:H].astype(np.float64)).astype(f32)
    cb[:, CB_FW1:CB_FW1 + 2048] = _pack_k_major(fw1_folded, H, 256).astype(bf16)
    cb[:, CB_FW2:CB_FW2 + 10] = _pack_k_major(0.5 * fc_w2, 256, 5).astype(bf16)
    fb1_eff = fb1_eff.astype(f32)
    cb[0, CB_FB1R:CB_FB1R + 128] = fb1_eff[0:128].astype(bf16)
    cb[0, CB_FB1R + 128:CB_FB1R + 256] = fb1_eff[128:256].astype(bf16)

    in_maps = []
    for i in range(NCORES):
        msk = np.zeros((BPC, S // 128, 128, 8), f32)
        cf_i = cf.copy()
        for b in range(BPC):
            gb = BPC * i + b
            for ty in range(4):
                msk[b, :, :, 2 * ty + b] = masks[ty][gb].reshape(S // 128, 128)
                cf_i[2 * ty + b, CF_INVC] = invs[ty][gb]
        wm = np.ascontiguousarray(
            msk.reshape(NK, 128, 8).transpose(1, 0, 2).reshape(128, NK * 8)
        ).astype(bf16)
        in_maps.append(
            dict(
                hid=np.ascontiguousarray(hid[BPC * i:BPC * (i + 1)]).astype(bf16),
                wm=wm,
                cb=cb,
                cf=cf_i,
            )
        )
    return in_maps


def kernel(**inputs):
    if "nc" not in _NC_CACHE:
        _NC_CACHE["nc"] = _build_nc()
    nc = _NC_CACHE["nc"]
    in_maps = _host_prep(inputs)
    res = run_bass_kernel_spmd(nc, in_maps, core_ids=list(range(NCORES)))
    out = np.empty((B, 5), np.float32)
    for i in range(NCORES):
        out[BPC * i:BPC * (i + 1)] = res.results[i]["out"].T
    return out


# revision 44
# speedup vs baseline: 2.5819x; 1.0024x over previous
"""Trainium2 Bass kernel for nn_DirectionalMultiHeadClassifier.

Data-parallel over 8 NeuronCores: each core handles 2 of the 16 samples.

Math per sample (mirrors the reference):
  - 4 masked means over S of hidden [S,H]: full attention_mask, and three
    position-range masks derived from L = mask.sum() (first/second/ending).
    Computed on-device as one PSUM-accumulated matmul:
        pooled4[8, H] += W_chunk[128, 8].T @ hidden_chunk[128, H]
    where W is a host-built 0/1 mask matrix (4 mask types x 2 samples) and
    the 1/count normalization is applied afterwards.
  - LayerNorm on the full-mask pooled vector; ln_g/ln_b are folded on the
    host into every consumer of the normalized vector (thr head w1/b1 and
    the fc pooled-part weights/bias), so the device only normalizes.
  - 4 small MLP heads (H->128 -> exact GELU -> 128->1). The scalar head
    outputs only feed the final classifier's last 4 input features, so the
    128->1 layer is folded into the classifier on the host:
        fc1 += gelu_h @ (0.5 * w2_h outer fc_w1[1024+h, :])
        fc_b1_eff = fc_b1 + sum_h b2_h * fc_w1[1024+h, :]
  - Final classifier (1028->256 -> exact GELU -> 256->5).
  Exact GELU is computed as 0.5*z*(1+erf(z/sqrt(2))) with the 0.5 folded
  into the following layer's weights.  Every linear bias is applied as a
  K=1 rank-1 matmul (bias_row outer ones) accumulated into PSUM, so the
  GELU needs just one Erf activation per layer.

Compute dtype: hidden/masks/weights stream through the PE in bf16 (masks
are exact 0/1 in bf16); all accumulation is f32 in PSUM.
"""

import ml_dtypes
import numpy as np

import concourse.bass as bass
import concourse.tile as tile
from bass_rust import add_dep_helper
from concourse import bacc, mybir
from concourse.bass_utils import run_bass_kernel_spmd

B, S, H = 16, 2048, 1024
NCORES = 8
BPC = B // NCORES          # samples per core
NK = BPC * (S // 128)      # 128-row contraction chunks per core
TS = 512                   # S rows per hidden DMA tile (1 MiB bf16)
NT = S // TS               # DMA tiles per sample
RS2 = 0.7071067811865476   # 1/sqrt(2)
LN_EPS = 1e-5
EPS = 1e-9
F32 = mybir.dt.float32
BF16 = mybir.dt.bfloat16
HEADS = ["esc", "res", "end", "thr"]

# packed bf16 const-block column offsets
CB_W1 = 0                  # 4 x [128, 1024]
CB_MH = 4096               # 4 x [128, 256]
CB_FW1 = 5120              # [128, 2048]
CB_FW2 = 7168              # [128, 10]
CB_B1R = 7178              # 4 x [1, 128] bias rows (row 0)
CB_FB1R = 7690             # 2 x [1, 128] fc bias rows (row 0)
CB_FB2R = 7946             # [1, 5] out bias row (row 0)
CB_ONES = 7951             # [1, 2] ones (row 0)
CB_COLS = 7953
# packed f32 const-block column offsets
CF_INVC = 0                # [8, 1]
CF_ID8 = 1                 # [8, 8]
CF_ZERO = 9                # [128, 1] zeros (activation bias)
CF_COLS = 10

_NC_CACHE = {}


def _build_nc():
    """Build the per-core Bass program (identical on all 8 cores)."""
    from contextlib import ExitStack

    nc = bacc.Bacc(
        "TRN2", target_bir_lowering=False, debug=False, num_devices=NCORES
    )
    dp = nc.declare_dram_parameter
    hid_d = dp("hid", [BPC, S, H], BF16, isOutput=False)
    wm_d = dp("wm", [128, NK * 8], BF16, isOutput=False)
    cb_d = dp("cb", [128, CB_COLS], BF16, isOutput=False)
    cf_d = dp("cf", [128, CF_COLS], F32, isOutput=False)
    out_d = dp("out", [5, BPC], F32, isOutput=True)

    with tile.TileContext(nc) as tc, ExitStack() as ctx:
        const = ctx.enter_context(tc.tile_pool(name="const", bufs=1))
        hidp = ctx.enter_context(tc.tile_pool(name="hidp", bufs=BPC * NT))
        work = ctx.enter_context(tc.tile_pool(name="work", bufs=1))
        psmain = ctx.enter_context(tc.tile_pool(name="psmain", bufs=1, space="PSUM"))
        pssm = ctx.enter_context(tc.tile_pool(name="pssm", bufs=1, space="PSUM"))

        # ACT table warm-up: touch the activation functions used later so the
        # ~1.3us/table loads overlap the initial DMAs instead of serializing
        # into the epilogue.
        ws_in = work.tile([1, 1], F32)
        ws_b = work.tile([1, 1], F32)
        ws_out = work.tile([1, 1], F32)
        nc.vector.memset(ws_in[:], 0.0)
        nc.vector.memset(ws_b[:], 0.0)
        for fn in (
            mybir.ActivationFunctionType.Gelu,
            mybir.ActivationFunctionType.Sqrt,
        ):
            nc.scalar.activation(out=ws_out[:], in_=ws_in[:], func=fn, bias=ws_b[:])

        # All large DMAs go on the single sync HWDGE ring, explicitly chained
        # so they transfer strictly in this order: wm, tile1..3, consts,
        # tile4.  Sequential transfers hand each tile over ASAP (concurrent
        # round-robin would delay the FIRST tile by 4x) and the params arrive
        # right before the epilogue needs them.
        wm_sb = const.tile([128, NK * 8], BF16, name="c_wm", tag="c_wm")
        cb_sb = const.tile([128, CB_COLS], BF16, name="c_cb", tag="c_cb")
        cf_sb = const.tile([128, CF_COLS], F32, name="c_cf", tag="c_cf")
        dma_chain = [
            nc.sync.dma_start(out=cf_sb[:], in_=cf_d[:]),
            nc.sync.dma_start(out=wm_sb[:], in_=wm_d[:]),
        ]

        # const views
        invc_v = cf_sb[0:8, CF_INVC:CF_INVC + 1]
        id8_v = cf_sb[0:8, CF_ID8:CF_ID8 + 8]
        i2_v = cf_sb[0:2, CF_ID8:CF_ID8 + 2]
        zero_v = cf_sb[:, CF_ZERO:CF_ZERO + 1]
        w1_v = lambda h, c: cb_sb[:, CB_W1 + 1024 * h + 128 * c:CB_W1 + 1024 * h + 128 * (c + 1)]
        mh_v = lambda h, m: cb_sb[:, CB_MH + 256 * h + 128 * m:CB_MH + 256 * h + 128 * (m + 1)]
        fw1_v = lambda c, m: cb_sb[:, CB_FW1 + 256 * c + 128 * m:CB_FW1 + 256 * c + 128 * (m + 1)]
        fw2_v = lambda m: cb_sb[:, CB_FW2 + 5 * m:CB_FW2 + 5 * (m + 1)]
        b1r_v = lambda h: cb_sb[0:1, CB_B1R + 128 * h:CB_B1R + 128 * (h + 1)]
        fb1r_v = lambda m: cb_sb[0:1, CB_FB1R + 128 * m:CB_FB1R + 128 * (m + 1)]
        fb2r_v = cb_sb[0:1, CB_FB2R:CB_FB2R + 5]
        ones_v = cb_sb[0:1, CB_ONES:CB_ONES + 2]

        # Wait-absorbers: every engine instruction carries at most ONE
        # semaphore wait in this walrus build, so consume each const DMA's
        # completion once per reading engine; real consumers then only wait
        # on their data inputs.
        scr_ps = pssm.tile([8, 8], F32)

        def absorb(csb):
            return nc.tensor.matmul(
                scr_ps[:, :], lhsT=csb[:, 0:8], rhs=csb[:, 0:8],
                start=True, stop=True,
            )

        # PE warm-up: the HAM clock gate defaults to 1.2 GHz and needs ~3.4us
        # of sustained activity to unthrottle.  Run junk matmuls during the
        # initial DMA wait so the real loop starts (and stays) at 2.4 GHz.
        warm_in = work.tile([128, 256], BF16)
        nc.vector.memset(warm_in[:], 0.0)
        warm_ps = pssm.tile([8, 512], F32)
        warm_last = None
        for _ in range(72):
            warm_last = nc.tensor.matmul(
                warm_ps[:, 0:256], lhsT=warm_in[:, 0:8], rhs=warm_in[:, 0:256],
                start=True, stop=True,
            )

        wm_abs = absorb(wm_sb)
        add_dep_helper(wm_abs.ins, warm_last.ins, sync=False, reason="warmup before wm absorber")

        # ---- main loop: pooled4[j, h] = sum_s wm[s, j] * hidden[s, h] ----
        pooled_ps = psmain.tile([8, H], F32)
        first_mm = None
        last_mm = None
        tiles = [(b, t) for b in range(BPC) for t in range(NT)]
        for k, (b, t) in enumerate(tiles):
            ht = hidp.tile([128, TS // 128, H], BF16)
            dma_chain.append(
                nc.sync.dma_start(
                    out=ht[:],
                    in_=hid_d[b, t * TS:(t + 1) * TS, :].rearrange(
                        "(c p) h -> p c h", p=128
                    ),
                )
            )
            for c in range(TS // 128):
                n = b * (S // 128) + t * (TS // 128) + c
                lw = wm_sb[:, n * 8:(n + 1) * 8]
                for j in range(2):
                    mm = nc.tensor.matmul(
                        pooled_ps[:, j * 512:(j + 1) * 512],
                        lhsT=lw,
                        rhs=ht[:, c, j * 512:(j + 1) * 512],
                        start=(n == 0),
                        stop=(n == NK - 1),
                    )
                    if first_mm is None:
                        first_mm = mm
                    last_mm = mm

        # the epilogue weight block transfers LAST: the whole pre-head part of
        # the epilogue (LayerNorm, transposes, relu inputs) doesn't need it,
        # so it streams in parallel with those stages.
        dma_chain.append(nc.sync.dma_start(out=cb_sb[:], in_=cb_d[:]))
        for k in range(1, len(dma_chain)):
            add_dep_helper(
                dma_chain[k].ins, dma_chain[k - 1].ins, sync=False,
                reason="serialize sync-ring DMAs",
            )
        add_dep_helper(first_mm.ins, wm_abs.ins, sync=False, reason="absorb wm dma wait")

        # absorbers/touches for epilogue consts; cf is tiny and arrives first
        # (absorb before the main loop), cb arrives last (absorb after it).
        cf_abs = absorb(cf_sb)
        add_dep_helper(cf_abs.ins, wm_abs.ins, sync=False, reason="cf absorber after warmup")
        add_dep_helper(first_mm.ins, cf_abs.ins, sync=False, reason="cf absorbed before main loop")
        cb_abs = absorb(cb_sb)
        add_dep_helper(cb_abs.ins, last_mm.ins, sync=False, reason="absorber after main loop")
        tv_cf = work.tile([1, 1], F32)
        t_cf = nc.vector.tensor_copy(tv_cf[0:1, 0:1], cf_sb[0:1, 0:1])
        ta_cf = work.tile([128, 1], F32)
        a_cf = nc.scalar.copy(out=ta_cf[:, 0:1], in_=cf_sb[:, 0:1])

        # ---- epilogue ----
        # Compute-engine APs must start at partition 0/32/64/96, so all
        # cross-row arithmetic happens after transposing to the free dim.
        # P4 rows: 0-1 pooled(s0,s1), 2-3 first, 4-5 second, 6-7 ending
        # The 1/count scaling runs on ACT (Copy with per-partition scale)
        # while DVE computes the LayerNorm stats straight from raw PSUM:
        # mu' = mu_raw*inv, rstd' = 1/sqrt(var_raw*inv^2 + eps), and
        # xn = (raw - mu_raw) * (inv * rstd').
        P4 = work.tile([8, H], F32)
        p4op = nc.scalar.activation(
            out=P4[:, 0:512], in_=pooled_ps[:, 0:512],
            func=mybir.ActivationFunctionType.Copy, bias=0.0, scale=invc_v,
        )
        add_dep_helper(p4op.ins, a_cf.ins, sync=False, reason="cf act touch first")
        p4op2 = nc.vector.tensor_scalar_mul(
            out=P4[:, 512:1024], in0=pooled_ps[:, 512:1024], scalar1=invc_v
        )
        add_dep_helper(p4op2.ins, t_cf.ins, sync=False, reason="cf touch first")

        stats = work.tile([2, 2, 6], F32)
        nc.vector.bn_stats(out=stats[:, 0, :], in_=pooled_ps[0:2, 0:512])
        nc.vector.bn_stats(out=stats[:, 1, :], in_=pooled_ps[0:2, 512:1024])
        mv = work.tile([2, 2], F32)
        bnop = nc.vector.bn_aggr(out=mv[:], in_=stats[:])
        add_dep_helper(bnop.ins, t_cf.ins, sync=False, reason="cf touch first")
        iv2 = work.tile([2, 1], F32)
        nc.vector.tensor_mul(iv2[:], invc_v[0:2, :], invc_v[0:2, :])
        vsc = work.tile([2, 1], F32)
        nc.vector.tensor_mul(vsc[:], mv[:, 1:2], iv2[:])
        eps_sb = work.tile([2, 1], F32)
        nc.vector.memset(eps_sb[:], LN_EPS)
        rstd = work.tile([2, 1], F32)
        sqop = nc.scalar.activation(
            out=rstd[:], in_=vsc[:],
            func=mybir.ActivationFunctionType.Sqrt, bias=eps_sb[:], scale=1.0,
        )
        # re-warm the Gelu table right after the (sole) Sqrt use so the later
        # Gelu activations don't pay the table load on the critical chain
        erf_rewarm = nc.scalar.activation(
            out=ws_out[:], in_=ws_in[:],
            func=mybir.ActivationFunctionType.Gelu, bias=ws_b[:],
        )
        add_dep_helper(erf_rewarm.ins, sqop.ins, sync=False, reason="erf rewarm after sqrt")
        nc.vector.reciprocal(rstd[:], rstd[:])
        mu2 = work.tile([2, 1], F32)
        nc.vector.tensor_mul(mu2[:], mv[:, 0:1], invc_v[0:2, :])
        xn = work.tile([2, H], F32)
        nc.vector.tensor_scalar(
            out=xn[:], in0=P4[0:2, :], scalar1=mu2[:], scalar2=rstd[:],
            op0=mybir.AluOpType.subtract, op1=mybir.AluOpType.mult,
        )

        # XTR[:, 10c + r]: r in 0..8 = P4 row r, r in 8..10 = xn row r-8,
        # for H positions c*128..(c+1)*128 on partitions.  The P4 transposes
        # and their cast run first so the esc/res/end heads are unblocked
        # before the LayerNorm (xn) path resolves.
        xtr_ps = pssm.tile([128, 80], F32)
        xtr_v = xtr_ps[:].rearrange("p (c r) -> p c r", r=10)
        XTR = work.tile([128, 8, 10], BF16)
        first_tr = None
        for cc in range(8):
            tr = nc.tensor.transpose(
                out=xtr_ps[:, cc * 10:cc * 10 + 8],
                in_=P4[:, cc * 128:(cc + 1) * 128],
                identity=id8_v,
            )
            if first_tr is None:
                first_tr = tr
                add_dep_helper(first_tr.ins, cf_abs.ins, sync=False, reason="cf absorbed before transposes")
        nc.vector.tensor_copy(XTR[:, :, 0:8], xtr_v[:, :, 0:8])

        # head inputs on the free dim: esc = relu(second-first), res = relu(-d)
        dT = work.tile([128, 8, 2], BF16)
        nc.vector.tensor_sub(dT[:], XTR[:, :, 4:6], XTR[:, :, 2:4])
        escT = work.tile([128, 8, 2], BF16)
        nc.vector.tensor_scalar_max(out=escT[:], in0=dT[:], scalar1=0.0)
        resT = work.tile([128, 8, 2], BF16)
        nc.vector.tensor_scalar(
            out=resT[:], in0=dT[:], scalar1=-1.0, scalar2=0.0,
            op0=mybir.AluOpType.mult, op1=mybir.AluOpType.max,
        )

        def head_rhs(h, cc):
            if h == 0:
                return escT[:, cc, :]
            if h == 1:
                return resT[:, cc, :]
            if h == 2:
                return XTR[:, cc, 6:8]
            return XTR[:, cc, 8:10]

        # head first layers: h1[:, 2h+j] = b1_h + w1_h.T @ x_{h,j}
        # esc/res/end run first (they don't depend on the LayerNorm path);
        # the xn transposes and the thr head follow.
        h1_ps = pssm.tile([128, 8], F32)
        for h in range(3):
            bmm = nc.tensor.matmul(
                h1_ps[:, 2 * h:2 * h + 2], lhsT=b1r_v(h), rhs=ones_v,
                start=True, stop=False,
            )
            if h == 0:
                add_dep_helper(bmm.ins, cb_abs.ins, sync=False, reason="cb absorbed before heads")
            for cc in range(8):
                nc.tensor.matmul(
                    h1_ps[:, 2 * h:2 * h + 2],
                    lhsT=w1_v(h, cc),
                    rhs=head_rhs(h, cc),
                    start=False,
                    stop=(cc == 7),
                )
        for cc in range(8):
            nc.tensor.transpose(
                out=xtr_ps[:, cc * 10 + 8:cc * 10 + 10],
                in_=xn[:, cc * 128:(cc + 1) * 128],
                identity=i2_v,
            )
        nc.vector.tensor_copy(XTR[:, :, 8:10], xtr_v[:, :, 8:10])
        nc.tensor.matmul(
            h1_ps[:, 6:8], lhsT=b1r_v(3), rhs=ones_v, start=True, stop=False,
        )
        for cc in range(8):
            nc.tensor.matmul(
                h1_ps[:, 6:8], lhsT=w1_v(3, cc), rhs=XTR[:, cc, 8:10],
                start=False, stop=(cc == 7),
            )
        g1 = work.tile([128, 8], BF16)
        g1op = nc.scalar.activation(
            out=g1[:], in_=h1_ps[:],
            func=mybir.ActivationFunctionType.Gelu, bias=zero_v, scale=1.0,
        )
        add_dep_helper(g1op.ins, erf_rewarm.ins, sync=False, reason="gelu rewarmed first")

        # fc1[:, 2m+j] = fb1 + fc_w1.T @ pooled_j + sum_h mh_h.T @ g1_{h,j}
        fc1_ps = pssm.tile([128, 4], F32)
        for m in range(2):
            sl = slice(2 * m, 2 * m + 2)
            nc.tensor.matmul(
                fc1_ps[:, sl], lhsT=fb1r_v(m), rhs=ones_v,
                start=True, stop=False,
            )
            for cc in range(8):
                nc.tensor.matmul(
                    fc1_ps[:, sl],
                    lhsT=fw1_v(cc, m),
                    rhs=XTR[:, cc, 8:10],
                    start=False,
                    stop=False,
                )
            for h in range(4):
                nc.tensor.matmul(
                    fc1_ps[:, sl],
                    lhsT=mh_v(h, m),
                    rhs=g1[:, 2 * h:2 * h + 2],
                    start=False,
                    stop=(h == 3),
                )
        g2 = work.tile([128, 4], BF16)
        nc.scalar.activation(
            out=g2[:], in_=fc1_ps[:],
            func=mybir.ActivationFunctionType.Gelu, bias=zero_v, scale=1.0,
        )

        out_ps = pssm.tile([5, 2], F32)
        nc.tensor.matmul(out_ps[:], lhsT=fb2r_v, rhs=ones_v, start=True, stop=False)
        for m in range(2):
            nc.tensor.matmul(
                out_ps[:],
                lhsT=fw2_v(m),
                rhs=g2[:, 2 * m:2 * m + 2],
                start=False,
                stop=(m == 1),
            )
        out_sb = work.tile([5, 2], F32)
        nc.vector.tensor_copy(out_sb[:], out_ps[:])
        nc.gpsimd.dma_start(out=out_d[:, :], in_=out_sb[:])

    nc.compile()
    return nc


def _pack_k_major(w, k, m):
    """[K, M] -> [128, (K//128)*M] with lhsT chunk c at cols [c*M, (c+1)*M)."""
    return np.ascontiguousarray(
        w.reshape(k // 128, 128, m).transpose(1, 0, 2).reshape(128, (k // 128) * m)
    ).astype(np.float32)


def _host_prep(inputs):
    """Build all per-core in_maps from the full inputs."""
    f32 = np.float32
    bf16 = ml_dtypes.bfloat16
    am = np.asarray(inputs["attention_mask"])
    hid = np.asarray(inputs["hidden"], dtype=f32)

    m_full = am.astype(f32)                      # [B, S]
    L = am.astype(np.int64).sum(1)               # [B]
    pos = np.arange(S)[None, :]
    mid = (L // 2)[:, None]
    Lb = L[:, None]
    st = np.maximum(1, L - 64)[:, None]
    fm = ((pos >= 1) & (pos < mid)).astype(f32)
    sm = ((pos >= mid) & (pos < Lb - 1)).astype(f32)
    em = ((pos >= st) & (pos < Lb - 1)).astype(f32)
    masks = [m_full, fm, sm, em]                 # type order: pooled,first,second,ending
    invs = [
        (1.0 / np.maximum(mk.sum(1, dtype=np.float64), EPS)).astype(f32)
        for mk in masks
    ]

    ln_g = np.asarray(inputs["ln_g"], np.float64)
    ln_b = np.asarray(inputs["ln_b"], np.float64)

    fc_w1 = np.asarray(inputs["fc_w1"], f32)     # [H+4, 256]
    fc_b1 = np.asarray(inputs["fc_b1"], f32)
    fc_w2 = np.asarray(inputs["fc_w2"], f32)     # [256, 5]
    fc_b2 = np.asarray(inputs["fc_b2"], f32)

    # packed const blocks
    cf = np.zeros((128, CF_COLS), f32)
    cf[0:8, CF_ID8:CF_ID8 + 8] = np.eye(8, dtype=f32)
    cb = np.zeros((128, CB_COLS), bf16)
    cb[0, CB_FB2R:CB_FB2R + 5] = fc_b2.astype(bf16)
    cb[0, CB_ONES:CB_ONES + 2] = np.ones(2, bf16)

    fb1_eff = fc_b1.astype(np.float64) + ln_b @ fc_w1[:H].astype(np.float64)
    for h, name in enumerate(HEADS):
        w1 = np.asarray(inputs[f"{name}_w1"], f32).astype(np.float64)  # [H, 128]
        b1 = np.asarray(inputs[f"{name}_b1"], f32).astype(np.float64)  # [128]
        w2 = np.asarray(inputs[f"{name}_w2"], f32)   # [128, 1]
        b2 = np.asarray(inputs[f"{name}_b2"], f32)   # [1]
        if name == "thr":
            # fold the LayerNorm affine into the thr head input weights
            b1 = b1 + ln_b @ w1
            w1 = ln_g[:, None] * w1
        cb[:, CB_W1 + 1024 * h:CB_W1 + 1024 * (h + 1)] = _pack_k_major(
            w1.astype(f32), H, 128
        ).astype(bf16)
        cb[0, CB_B1R + 128 * h:CB_B1R + 128 * (h + 1)] = b1.astype(bf16)
        cb[:, CB_MH + 256 * h:CB_MH + 256 * (h + 1)] = np.ascontiguousarray(
            w2[:, 0][:, None] * fc_w1[H + h, :][None, :]
        ).astype(bf16)
        fb1_eff = fb1_eff + b2[0] * fc_w1[H + h, :].astype(np.float64)

    fw1_folded = (ln_g[:, None] * fc_w1[:H].astype(np.float64)).astype(f32)
    cb[:, CB_FW1:CB_FW1 + 2048] = _pack_k_major(fw1_folded, H, 256).astype(bf16)
    cb[:, CB_FW2:CB_FW2 + 10] = _pack_k_major(fc_w2, 256, 5).astype(bf16)
    fb1_eff = fb1_eff.astype(f32)
    cb[0, CB_FB1R:CB_FB1R + 128] = fb1_eff[0:128].astype(bf16)
    cb[0, CB_FB1R + 128:CB_FB1R + 256] = fb1_eff[128:256].astype(bf16)

    in_maps = []
    for i in range(NCORES):
        msk = np.zeros((BPC, S // 128, 128, 8), f32)
        cf_i = cf.copy()
        for b in range(BPC):
            gb = BPC * i + b
            for ty in range(4):
                msk[b, :, :, 2 * ty + b] = masks[ty][gb].reshape(S // 128, 128)
                cf_i[2 * ty + b, CF_INVC] = invs[ty][gb]
        wm = np.ascontiguousarray(
            msk.reshape(NK, 128, 8).transpose(1, 0, 2).reshape(128, NK * 8)
        ).astype(bf16)
        in_maps.append(
            dict(
                hid=np.ascontiguousarray(hid[BPC * i:BPC * (i + 1)]).astype(bf16),
                wm=wm,
                cb=cb,
                cf=cf_i,
            )
        )
    return in_maps


def kernel(**inputs):
    if "nc" not in _NC_CACHE:
        _NC_CACHE["nc"] = _build_nc()
    nc = _NC_CACHE["nc"]
    in_maps = _host_prep(inputs)
    res = run_bass_kernel_spmd(nc, in_maps, core_ids=list(range(NCORES)))
    out = np.empty((B, 5), np.float32)
    for i in range(NCORES):
        out[BPC * i:BPC * (i + 1)] = res.results[i]["out"].T
    return out
